# revision 1
# baseline (speedup 1.0000x reference)
"""NodeNet GNN message-passing kernel for 8 Trainium2 NeuronCores.

Strategy (per sharding hint): shard nodes across the 8 cores; partition
edges by destination node on the host so the scatter-mean is device-local.

Per core (12,500 real nodes, padded to 12,544 = 196 windows of 64 nodes):
  - Host sorts edges by destination and pre-scales each edge row by
    1/count(dst), so the device segment-sum directly yields the mean.
    Each 64-node window's edge list is padded to a multiple of 128; each
    core processes its windows in descending-edge-count order so the
    shared (SPMD) per-window chunk counts CB[j] = max-over-cores of the
    j-th order statistic waste minimal padding, and the smallest windows
    land at the end, shortening the pipeline drain.  Edge features are
    laid out chunk-transposed fp16 ([128, nch*128]), with each MLP
    group's node features interleaved into the same stream, so the whole
    input arrives as one wide contiguous DMA per group.
  - Device builds, per 128-edge chunk, a [128 edge, 64 node] fp16
    one-hot (is_equal of dst-rel against an iota ramp, VectorE) and
    contracts it on the TensorEngine:
    meanT[d, n] += matmul(lhsT=attr[e, d], rhs=onehot[e, n]) accumulated
    in PSUM (fp32).  Everything stays feature-major so the 3-layer MLP
    (fp16 matmuls, fp32 PSUM accumulate, ScalarE relu+bias evacuations)
    chains with no transposes: h1T = relu(W1.T @ [xT; meanT] + b1), ...
  - Windows whose (cross-core max) edge-count remainder fits in 64 edges
    pair up inside each group: two tails share one 128-row chunk (one in
    partitions 0:64, the other in 64:128, contracted by two K=64
    matmuls), trimming most of the chunk-quantization padding.
  - Output is accumulated feature-major fp16 in SBUF and stored with one
    deferred DMA per group; the host transposes, upcasts, and un-permutes.

Cost-model timeline (per core): ~182 us against a ~169 us DMA-byte
floor (~61 MB/core at ~360 GB/s); VectorE/ScalarE/TensorE all at or
below ~65% occupancy, fully hidden behind the edge-feature stream.
"""

import numpy as np

import concourse.bacc as bacc
import concourse.mybir as mybir
import concourse.tile as tile
from concourse.bass_utils import run_bass_kernel_spmd

P = 128                    # partitions / matmul contraction tile
D = 128                    # node & edge feature dim
HIDDEN = 256
DOUT = 128
N_NODES = 100000
N_CORES = 8
NPC_REAL = 12500           # real nodes per core
W = 64                     # nodes per binning window
WINDOWS = 196              # windows per core (196*64 = 12544)
NPC = WINDOWS * W          # padded nodes per core
GPW = 8                    # windows per MLP group (512 nodes)
GROUP_N = GPW * W
ATTR_BUFS = 3
OH_BUFS = 7
ACT_BUFS = 4
PBIN_BUFS = 4

_prog_cache: dict = {}

f32 = mybir.dt.float32
f16 = mybir.dt.float16


def _group_sizes():
    # a small first group lets compute start ~4us earlier while the
    # pipeline ramps; full groups in steady state; taper at the tail
    gsizes = [2]
    rem = WINDOWS - 2
    while rem > GPW:
        gsizes.append(GPW)
        rem -= GPW
    while rem > 0:
        t = min(GPW // 2, rem)
        gsizes.append(t)
        rem -= t
    return gsizes


def _build_program(META, ablate=()):
    """Build the Bass/Tile program. META = (NCH, per-window tuples of
    (col_off, ncols, fullc, tailmode)) — identical across cores.
    tailmode: 0 = all-full chunks; 1 = last chunk is a shared half
    (partitions 0:64); 2 = first chunk is a shared half (partitions
    64:128).  ablate: subset of {"mlp", "bin", "oh"} (sim studies)."""
    NCH, wmeta = META
    col_off = [m[0] for m in wmeta]
    ncols = [m[1] for m in wmeta]
    CBmax = max(ncols)

    nc = bacc.Bacc(None)
    # attrT carries, per group: the edge-feature chunks, then the group's
    # node features (gsz*W fp16 columns) — one combined DMA per group.
    attrT_d = nc.dram_tensor(
        "attrT", [P, NCH * D + WINDOWS * W], f16, kind="ExternalInput"
    )
    # fp16 consts: iota ramp (CBmax*W) | dstrel (NCH)
    c16_d = nc.dram_tensor("c16", [P, CBmax * W + NCH], f16, kind="ExternalInput")
    # fp32 consts: 5 bias columns
    consts_d = nc.dram_tensor("consts", [P, 5], f32, kind="ExternalInput")
    wts_d = nc.dram_tensor("wts", [P, 4 * HIDDEN + 2 * DOUT], f16,
                           kind="ExternalInput")
    outT_d = nc.dram_tensor("outT", [P, NPC], f16, kind="ExternalOutput")

    Relu = mybir.ActivationFunctionType.Relu
    Ident = mybir.ActivationFunctionType.Identity

    with tile.TileContext(nc) as tc:
        with (
            tc.tile_pool(name="const", bufs=1) as cpool,
            tc.tile_pool(name="attr", bufs=ATTR_BUFS) as apool,
            tc.tile_pool(name="oh", bufs=OH_BUFS) as ohpool,
            tc.tile_pool(name="acts", bufs=ACT_BUFS) as actpool,
            tc.tile_pool(name="pbin", bufs=PBIN_BUFS, space="PSUM") as pbin,
            tc.tile_pool(name="pmlp", bufs=1, space="PSUM") as pmlp,
        ):
            # --- constants (tiles now; DMAs after the first attr DMA so
            # the edge stream starts immediately) ---
            cs = cpool.tile([P, 5], f32, tag="consts")
            ws = cpool.tile([P, 4 * HIDDEN + 2 * DOUT], f16, tag="wts")
            c16 = cpool.tile([P, CBmax * W + NCH], f16, tag="c16")
            w1s_0 = ws[:, 0:HIDDEN]
            w1s_1 = ws[:, HIDDEN : 2 * HIDDEN]
            w2s_0 = ws[:, 2 * HIDDEN : 3 * HIDDEN]
            w2s_1 = ws[:, 3 * HIDDEN : 4 * HIDDEN]
            w3s_0 = ws[:, 4 * HIDDEN : 4 * HIDDEN + DOUT]
            w3s_1 = ws[:, 4 * HIDDEN + DOUT : 4 * HIDDEN + 2 * DOUT]
            b1s_0 = cs[:, 0:1]
            b1s_1 = cs[:, 1:2]
            b2s_0 = cs[:, 2:3]
            b2s_1 = cs[:, 3:4]
            b3s = cs[:, 4:5]
            it16 = c16[:, 0 : CBmax * W]
            dstrel_s = c16[:, CBmax * W : CBmax * W + NCH]
            oall = cpool.tile([P, NPC], f16, tag="oall")

            # group sizes: GPW windows each, tapering at the tail to
            # shorten the pipeline drain (last windows are also the
            # smallest thanks to the descending-count permutation)
            gsizes = _group_sizes()
            gstart = [0]
            for s in gsizes:
                gstart.append(gstart[-1] + s)

            for j in range(WINDOWS):
                off, cb, fullc, tmode = wmeta[j]
                g = next(i for i in range(len(gsizes)) if gstart[i + 1] > j)
                sw = j - gstart[g]
                gsz = gsizes[g]

                if sw == 0:
                    # one combined edge-feature + node-feature DMA per group
                    goff = off
                    jl = gstart[g + 1] - 1
                    gend = int(wmeta[jl][0] + wmeta[jl][1])
                    gw = (gend - goff) * D + gsz * W
                    gsrc = goff * D + gstart[g] * W
                    at = apool.tile([P, GPW * (CBmax * D + W)], f16, tag="attr")
                    nc.sync.dma_start(
                        out=at[:, :gw], in_=attrT_d[:, gsrc : gsrc + gw]
                    )
                    if j == 0:
                        nc.sync.dma_start(out=c16[:], in_=c16_d[:, :])
                        nc.sync.dma_start(out=cs[:], in_=consts_d[:, :])
                        nc.sync.dma_start(out=ws[:], in_=wts_d[:, :])
                    # flush the previous group's finished output slice
                    if g > 0 and gsizes[g - 1] == GPW:
                        f0, f1 = gstart[g - 1] * W, gstart[g] * W
                        nc.sync.dma_start(
                            out=outT_d[:, f0:f1], in_=oall[:, f0:f1]
                        )
                woff = off - goff  # window's chunk offset within group tile

                oh = ohpool.tile([P, CBmax * W], f16, tag="oh")
                if "oh" not in ablate:
                    nc.vector.tensor_tensor(
                        out=oh[:, : cb * W].rearrange("p (c m) -> p c m", m=W),
                        in0=dstrel_s[:, off : off + cb].to_broadcast([P, cb, W]),
                        in1=it16[:, : cb * W].rearrange("p (c m) -> p c m", m=W),
                        op=mybir.AluOpType.is_equal,
                    )

                pm = pbin.tile([P, W], f32, tag="mean")
                if "bin" not in ablate:
                    # (chunk-index-in-window, partition range) per matmul
                    if tmode == 1:      # shared half chunk last, rows 0:64
                        parts = [(ch, 0, P) for ch in range(fullc)]
                        parts.append((fullc, 0, 64))
                    elif tmode == 2:    # shared half chunk first, rows 64:128
                        parts = [(0, 64, P)]
                        parts += [(ch, 0, P) for ch in range(1, cb)]
                    else:
                        parts = [(ch, 0, P) for ch in range(cb)]
                    for i, (ch, p0, p1) in enumerate(parts):
                        nc.tensor.matmul(
                            out=pm[:],
                            lhsT=at[p0:p1, (woff + ch) * D : (woff + ch + 1) * D],
                            rhs=oh[p0:p1, ch * W : (ch + 1) * W],
                            start=(i == 0),
                            stop=(i == len(parts) - 1),
                        )

                if sw == 0:
                    mean_g = actpool.tile([P, GROUP_N], f16, tag="mean_g")
                if "bin" not in ablate:
                    nc.scalar.copy(out=mean_g[:, sw * W : (sw + 1) * W], in_=pm[:])

                if ("mlp" not in ablate) and (sw == gsz - 1):
                    # --- MLP over this group of nodes (feature-major) ---
                    NW = gsz * W
                    n0 = gstart[g] * W

                    ph1a = pmlp.tile([P, GROUP_N], f32, tag="h1a")
                    ph1b = pmlp.tile([P, GROUP_N], f32, tag="h1b")
                    nc.tensor.matmul(out=ph1a[:, :NW], lhsT=w1s_0[:, 0:P],
                                     rhs=at[:, (gend - goff) * D : (gend - goff) * D + NW], start=True, stop=False)
                    nc.tensor.matmul(out=ph1a[:, :NW], lhsT=w1s_1[:, 0:P],
                                     rhs=mean_g[:, :NW], start=False, stop=True)
                    nc.tensor.matmul(out=ph1b[:, :NW], lhsT=w1s_0[:, P:HIDDEN],
                                     rhs=at[:, (gend - goff) * D : (gend - goff) * D + NW], start=True, stop=False)
                    nc.tensor.matmul(out=ph1b[:, :NW], lhsT=w1s_1[:, P:HIDDEN],
                                     rhs=mean_g[:, :NW], start=False, stop=True)
                    h1a = actpool.tile([P, GROUP_N], f16, tag="h1a_s")
                    h1b = actpool.tile([P, GROUP_N], f16, tag="h1b_s")
                    nc.scalar.activation(out=h1a[:, :NW], in_=ph1a[:, :NW],
                                         func=Relu, bias=b1s_0[:, 0:1])
                    nc.scalar.activation(out=h1b[:, :NW], in_=ph1b[:, :NW],
                                         func=Relu, bias=b1s_1[:, 0:1])

                    ph2a = pmlp.tile([P, GROUP_N], f32, tag="h2a")
                    ph2b = pmlp.tile([P, GROUP_N], f32, tag="h2b")
                    nc.tensor.matmul(out=ph2a[:, :NW], lhsT=w2s_0[:, 0:P],
                                     rhs=h1a[:, :NW], start=True, stop=False)
                    nc.tensor.matmul(out=ph2a[:, :NW], lhsT=w2s_1[:, 0:P],
                                     rhs=h1b[:, :NW], start=False, stop=True)
                    nc.tensor.matmul(out=ph2b[:, :NW], lhsT=w2s_0[:, P:HIDDEN],
                                     rhs=h1a[:, :NW], start=True, stop=False)
                    nc.tensor.matmul(out=ph2b[:, :NW], lhsT=w2s_1[:, P:HIDDEN],
                                     rhs=h1b[:, :NW], start=False, stop=True)
                    h2a = actpool.tile([P, GROUP_N], f16, tag="h2a_s")
                    h2b = actpool.tile([P, GROUP_N], f16, tag="h2b_s")
                    nc.scalar.activation(out=h2a[:, :NW], in_=ph2a[:, :NW],
                                         func=Relu, bias=b2s_0[:, 0:1])
                    nc.scalar.activation(out=h2b[:, :NW], in_=ph2b[:, :NW],
                                         func=Relu, bias=b2s_1[:, 0:1])

                    po = pmlp.tile([P, GROUP_N], f32, tag="h1a")
                    nc.tensor.matmul(out=po[:, :NW], lhsT=w3s_0[:],
                                     rhs=h2a[:, :NW], start=True, stop=False)
                    nc.tensor.matmul(out=po[:, :NW], lhsT=w3s_1[:],
                                     rhs=h2b[:, :NW], start=False, stop=True)
                    nc.scalar.activation(out=oall[:, n0 : n0 + NW],
                                         in_=po[:, :NW],
                                         func=Ident, bias=b3s[:, 0:1])
                    if gsz < GPW:
                        # tail taper groups: no more prefetches to protect,
                        # store immediately to shorten the drain
                        nc.sync.dma_start(
                            out=outT_d[:, n0 : n0 + NW], in_=oall[:, n0 : n0 + NW]
                        )

            if gsizes[-1] == GPW:
                f0 = gstart[len(gsizes) - 1] * W
                nc.sync.dma_start(out=outT_d[:, f0:], in_=oall[:, f0:])

    # run_bass_via_pjrt (axon path) does not finalize; Bacc needs
    # finalize() to run its compile passes (reg alloc, wait legalization).
    nc.finalize()
    return nc


def _host_prep(x, edge_index, edge_attr):
    """Sort/scale/pad edges; returns (CB, per-core input arrays)."""
    col = np.asarray(edge_index)[1].astype(np.int64)
    x = np.asarray(x, dtype=np.float32)
    counts = np.bincount(col, minlength=N_NODES)
    scale = (1.0 / np.maximum(counts, 1)).astype(np.float32)

    order = np.argsort(col, kind="stable")
    col_s = col[order]
    attr_s = np.asarray(edge_attr, dtype=np.float32)[order]
    attr_s = attr_s * scale[col_s][:, None]

    # per-core, per-window edge counts
    starts = np.empty((N_CORES, WINDOWS + 1), dtype=np.int64)
    for c in range(N_CORES):
        bounds = np.minimum(
            c * NPC_REAL + np.arange(WINDOWS + 1) * W, (c + 1) * NPC_REAL
        )
        starts[c] = np.searchsorted(col_s, bounds)
    cnt = np.diff(starts, axis=1)  # [N_CORES, WINDOWS]

    # Each core processes its windows sorted by descending edge count.
    # Window slot j then holds every core's j-th order statistic, so the
    # cross-core max (the chunk plan must be shared, the program is SPMD)
    # wastes far less padding than positional assignment.  Small windows
    # land last, which also shortens the pipeline drain.  Host un-permutes
    # outputs.
    order = np.argsort(-cnt, axis=1, kind="stable")  # [N_CORES, WINDOWS]
    cnt_s = np.take_along_axis(cnt, order, axis=1)

    # Shared tail chunks: windows whose (cross-core max) remainder fits in
    # 64 edges can pair up, two tails sharing one 128-row chunk (A in
    # partitions 0:64, B in 64:128).  Reorder slots inside each group so
    # tailable windows are adjacent; odd leftovers get promoted to a full
    # chunk.
    m = cnt_s.max(axis=0)
    fullc = (m // P).astype(np.int64)
    rem = m - fullc * P
    fullc += rem > 64                     # big remainders stay full chunks
    tailable = ((rem > 0) & (rem <= 64)) | (m == 0)

    gsz_list = _group_sizes()
    slot_perm = []
    tmode = np.zeros(WINDOWS, np.int64)   # 0 none, 1 A(rows 0:64), 2 B(64:128)
    pos = 0
    for gs in gsz_list:
        idx = np.arange(pos, pos + gs)
        tl = idx[tailable[idx]]
        nont = idx[~tailable[idx]]
        if len(tl) % 2 == 1:              # promote one leftover tail
            lone = tl[-1]
            tl = tl[:-1]
            fullc[lone] += (rem[lone] > 0) | (m[lone] == 0)
            nont = np.append(nont, lone)
        slot_perm.extend(nont.tolist())
        slot_perm.extend(tl.tolist())
        tmode[pos + len(nont) : pos + gs] = np.tile([1, 2], len(tl) // 2)
        pos += gs
    slot_perm = np.asarray(slot_perm)
    fullc = fullc[slot_perm]
    order = order[:, slot_perm]
    cnt_s = cnt_s[:, slot_perm]

    # column offsets: A's shared column is also B's first column
    col_off = np.zeros(WINDOWS, np.int64)
    ncols = np.zeros(WINDOWS, np.int64)
    co = 0
    for j in range(WINDOWS):
        if tmode[j] == 2:
            col_off[j] = co - 1
            ncols[j] = fullc[j] + 1
            co += fullc[j]
        elif tmode[j] == 1:
            col_off[j] = co
            ncols[j] = fullc[j] + 1
            co += fullc[j] + 1
        else:
            col_off[j] = co
            ncols[j] = fullc[j]
            co += fullc[j]
    NCH = int(co)
    E_pad = NCH * P
    wmeta = tuple(
        (int(col_off[j]), int(ncols[j]), int(fullc[j]), int(tmode[j]))
        for j in range(WINDOWS)
    )

    per_core = []
    for c in range(N_CORES):
        ordc = order[c]
        cnts = cnt_s[c]                      # counts in processing order
        total = int(cnts.sum())
        # edge source rows (into col_s/attr_s), in processing order
        src_idx = np.concatenate(
            [np.arange(starts[c, w], starts[c, w + 1]) for w in ordc]
        )
        within = np.arange(total) - np.repeat(np.cumsum(cnts) - cnts, cnts)
        co_e = np.repeat(col_off, cnts)
        fc_e = np.repeat(fullc, cnts)
        tm_e = np.repeat(tmode, cnts)
        # rows: mode 0/1 fill columns contiguously (tail rows start at row
        # 0 of the last column); mode B fills its full columns (one past
        # the shared one) first, remainder into rows 64: of the shared.
        edest = co_e * P + within
        isB = tm_e == 2
        infull = within < fc_e * P
        edest[isB & infull] = (co_e * P + P + within)[isB & infull]
        edest[isB & ~infull] = (co_e * P + 64 + (within - fc_e * P))[
            isB & ~infull
        ]

        attr_pad = np.zeros((E_pad, D), np.float32)
        attr_pad[edest] = attr_s[src_idx]
        attrT_edges = (
            attr_pad.reshape(NCH, P, D)
            .transpose(1, 0, 2)
            .reshape(P, NCH * D)
            .astype(np.float16)
        )

        # dst relative to the processed window's node base
        win_base_proc = c * NPC_REAL + ordc * W  # global node base per slot
        dstrel = np.full((E_pad,), 200.0, np.float16)
        dstrel[edest] = (
            col_s[src_idx] - np.repeat(win_base_proc, cnts)
        ).astype(np.float16)
        dstrelT = np.ascontiguousarray(dstrel.reshape(NCH, P).T)

        # node features per 64-node window slot, zero-padded per slot
        xc = np.zeros((WINDOWS, W, D), np.float16)
        for j, w in enumerate(ordc):
            n0 = c * NPC_REAL + w * W
            n1 = min(n0 + W, (c + 1) * NPC_REAL)
            xc[j, : n1 - n0] = x[n0:n1].astype(np.float16)
        xT = xc.reshape(NPC, D).T  # [D, NPC]

        # interleave per group: [edge chunks | node features]
        gsizes = _group_sizes()
        attrT = np.empty((P, NCH * D + WINDOWS * W), np.float16)
        pos = 0
        j0 = 0
        for gsz in gsizes:
            c0 = int(col_off[j0])
            c1 = int(col_off[j0 + gsz - 1] + ncols[j0 + gsz - 1])
            wgt = (c1 - c0) * D
            attrT[:, pos : pos + wgt] = attrT_edges[:, c0 * D : c1 * D]
            pos += wgt
            attrT[:, pos : pos + gsz * W] = xT[:, j0 * W : (j0 + gsz) * W]
            pos += gsz * W
            j0 += gsz
        assert pos == attrT.shape[1] and j0 == WINDOWS

        per_core.append(
            {"attrT": np.ascontiguousarray(attrT), "dstrelT": dstrelT,
             "order": ordc}
        )
    return (NCH, wmeta), per_core


def _build_consts(b1, b2, b3):
    consts = np.zeros((P, 5), np.float32)
    consts[:, 0] = b1[:P]
    consts[:, 1] = b1[P:]
    consts[:, 2] = b2[:P]
    consts[:, 3] = b2[P:]
    consts[:, 4] = b3
    return consts


def _build_wts(W1, W2, W3):
    wts = np.empty((P, 4 * HIDDEN + 2 * DOUT), np.float16)
    wts[:, 0:HIDDEN] = W1[:P]
    wts[:, HIDDEN : 2 * HIDDEN] = W1[P:]
    wts[:, 2 * HIDDEN : 3 * HIDDEN] = W2[:P]
    wts[:, 3 * HIDDEN : 4 * HIDDEN] = W2[P:]
    wts[:, 4 * HIDDEN : 4 * HIDDEN + DOUT] = W3[:P]
    wts[:, 4 * HIDDEN + DOUT : 4 * HIDDEN + 2 * DOUT] = W3[P:]
    return wts


def _build_c16(META, dstrelT):
    """fp16 consts row-block: iota ramp | dstrel."""
    NCH, wmeta = META
    CBmax = max(mw[1] for mw in wmeta)
    c16 = np.empty((P, CBmax * W + NCH), np.float16)
    c16[:, 0 : CBmax * W] = np.tile(np.arange(W, dtype=np.float16), CBmax)[None, :]
    c16[:, CBmax * W :] = dstrelT
    return c16


def kernel(x, edge_index, edge_attr, W1, b1, W2, b2, W3, b3):
    CB, per_core = _host_prep(x, edge_index, edge_attr)

    key = CB
    if key not in _prog_cache:
        _prog_cache[key] = _build_program(CB)
    nc = _prog_cache[key]

    W1 = np.asarray(W1, np.float32)
    W2 = np.asarray(W2, np.float32)
    W3 = np.asarray(W3, np.float32)
    b1 = np.asarray(b1, np.float32)
    b2 = np.asarray(b2, np.float32)
    b3 = np.asarray(b3, np.float32)
    consts = _build_consts(b1, b2, b3)
    wts = _build_wts(W1, W2, W3)
    in_maps = [
        {
            "attrT": pc["attrT"],
            "c16": _build_c16(CB, pc["dstrelT"]),
            "consts": consts,
            "wts": wts,
        }
        for pc in per_core
    ]

    res = run_bass_kernel_spmd(nc, in_maps, core_ids=list(range(N_CORES)))

    out = np.empty((N_NODES, DOUT), np.float32)
    for c in range(N_CORES):
        o = res.results[c]["outT"].T.astype(np.float32).reshape(WINDOWS, W, DOUT)
        for j, w in enumerate(per_core[c]["order"]):
            n0 = c * NPC_REAL + int(w) * W
            n1 = min(n0 + W, (c + 1) * NPC_REAL)
            out[n0:n1] = o[j, : n1 - n0]
    return out



# revision 5
# speedup vs baseline: 1.2763x; 1.2763x over previous
"""NodeNet GNN message-passing kernel for 8 Trainium2 NeuronCores.

Strategy (per sharding hint): shard nodes across the 8 cores; partition
edges by destination node on the host so the scatter-mean is device-local.

v2 — fp8 edge stream + padding-minimizing window packing:
  - Host sorts each core's 12,500 nodes by descending edge count and sorts
    edges by destination; edge rows are pre-scaled by 1/count(dst) so the
    device segment-sum directly yields the mean, then cast to fp8 e4m3
    (absmax error ~9e-3 vs the 2e-2 gate; the scatter-mean averages the
    quantization noise before the MLP sees it).
  - Windows hold up to W=16 nodes but close early so their edge lists land
    near 128-edge chunk boundaries (host DP over node ranks); the chunk
    plan is shared across cores (SPMD) and built from the rank-wise max
    count profile, so every core's greedy fill fits the plan.
  - Device builds, per group of windows, ONE batched [128 edge, chunk, W]
    fp16 one-hot (is_equal of dst-rel against an iota ramp, VectorE) and
    contracts chunk-by-chunk on the TensorEngine into per-window column
    slices of ONE per-group PSUM bank tile (fp8 lhsT x fp16 rhs, fp32
    accumulate) — one ScalarE evacuation per group instead of per window.
  - The 3-layer MLP runs feature-major per group (fp16 matmuls, fp32 PSUM)
    with PSUM evacuations split between ScalarE (mean, h1a, h1b, h2a) and
    VectorE (h2b, out) to keep both under the DMA roofline.

Cost-model timeline (per core): ~95 us DMA floor (~34 MB/core at 360
GB/s), PE ~67 us, ACT ~64 us, DVE ~61 us.
"""

import numpy as np
import ml_dtypes

import concourse.bacc as bacc
import concourse.mybir as mybir
import concourse.tile as tile
from concourse.bass_utils import run_bass_kernel_spmd

P = 128                    # partitions / matmul contraction tile
D = 128                    # node & edge feature dim
HIDDEN = 256
DOUT = 128
N_NODES = 100000
N_CORES = 8
NPC_REAL = 12500           # real nodes per core
W = 16                     # node slots per window (one-hot width)

# marginal cost weights for the host packing DP (ns, from the TRN2 cost
# model): one 128-edge chunk costs DMA 45.5 + PE 6.7 + DVE 16.7; one
# window costs 16 node slots of MLP/DMA work
CHUNK_COST = 70.0
WINDOW_COST = 176.0

_prog_cache: dict = {}

f32 = mybir.dt.float32
f16 = mybir.dt.float16
f8 = mybir.dt.float8e4

Relu = mybir.ActivationFunctionType.Relu
Ident = mybir.ActivationFunctionType.Identity


def _group_plan(n_windows):
    """Group sizes in windows: small groups first (compute starts early),
    steady-state 32-window groups (512 node slots), tapered tail."""
    gsizes = [8, 16, 24]
    rem = n_windows - sum(gsizes)
    while rem > 48:
        gsizes.append(32)
        rem -= 32
    for t in (24, 16, 8):
        while rem >= t:
            gsizes.append(t)
            rem -= t
    if rem:
        gsizes.append(rem)
    return gsizes


def _build_program(META):
    """META = (wplan, gbounds) with wplan a tuple of (chunk_off, ncols)
    per window and gbounds a tuple of (w0, w1) per group; identical
    across cores (SPMD)."""
    wplan, gbounds = META
    n_windows = len(wplan)
    NPC = n_windows * W
    NCH = wplan[-1][0] + wplan[-1][1]
    CBG_max = max(
        wplan[w1 - 1][0] + wplan[w1 - 1][1] - wplan[w0][0] for w0, w1 in gbounds
    )
    NW_max = max(w1 - w0 for w0, w1 in gbounds) * W

    nc = bacc.Bacc(None)
    attr8_d = nc.dram_tensor("attr8", [P, NCH * D], f8, kind="ExternalInput")
    x16_d = nc.dram_tensor("x16", [P, NPC], f16, kind="ExternalInput")
    # fp16 consts: iota ramp (CBG_max*W) | dstrel (NCH)
    c16_d = nc.dram_tensor("c16", [P, CBG_max * W + NCH], f16, kind="ExternalInput")
    consts_d = nc.dram_tensor("consts", [P, 5], f32, kind="ExternalInput")
    wts_d = nc.dram_tensor("wts", [P, 4 * HIDDEN + 2 * DOUT], f16,
                           kind="ExternalInput")
    outT_d = nc.dram_tensor("outT", [P, NPC], f16, kind="ExternalOutput")

    with tile.TileContext(nc) as tc:
        with (
            tc.tile_pool(name="const", bufs=1) as cpool,
            tc.tile_pool(name="attr", bufs=3) as apool,
            tc.tile_pool(name="x", bufs=2) as xpool,
            tc.tile_pool(name="oh", bufs=3) as ohpool,
            tc.tile_pool(name="acts", bufs=2) as actpool,
            tc.tile_pool(name="pbin", bufs=2, space="PSUM") as pbin,
            tc.tile_pool(name="pmlp", bufs=1, space="PSUM") as pmlp,
        ):
            cs = cpool.tile([P, 5], f32, tag="consts")
            ws = cpool.tile([P, 4 * HIDDEN + 2 * DOUT], f16, tag="wts")
            c16 = cpool.tile([P, CBG_max * W + NCH], f16, tag="c16")
            w1s_0 = ws[:, 0:HIDDEN]
            w1s_1 = ws[:, HIDDEN : 2 * HIDDEN]
            w2s_0 = ws[:, 2 * HIDDEN : 3 * HIDDEN]
            w2s_1 = ws[:, 3 * HIDDEN : 4 * HIDDEN]
            w3s_0 = ws[:, 4 * HIDDEN : 4 * HIDDEN + DOUT]
            w3s_1 = ws[:, 4 * HIDDEN + DOUT : 4 * HIDDEN + 2 * DOUT]
            b1s_0 = cs[:, 0:1]
            b1s_1 = cs[:, 1:2]
            b2s_0 = cs[:, 2:3]
            b2s_1 = cs[:, 3:4]
            b3s = cs[:, 4:5]
            it16 = c16[:, 0 : CBG_max * W]
            dstrel_s = c16[:, CBG_max * W : CBG_max * W + NCH]

            for g, (w0, w1) in enumerate(gbounds):
                NW = (w1 - w0) * W
                n0 = w0 * W
                co0 = wplan[w0][0]
                co1 = wplan[w1 - 1][0] + wplan[w1 - 1][1]
                CBg = co1 - co0

                at8 = apool.tile([P, CBG_max * D], f8, tag="attr")
                nc.sync.dma_start(
                    out=at8[:, : CBg * D], in_=attr8_d[:, co0 * D : co1 * D]
                )
                if g == 0:
                    nc.sync.dma_start(out=c16[:], in_=c16_d[:, :])
                    nc.sync.dma_start(out=cs[:], in_=consts_d[:, :])
                    nc.sync.dma_start(out=ws[:], in_=wts_d[:, :])
                xg = xpool.tile([P, NW_max], f16, tag="x")
                nc.sync.dma_start(out=xg[:, :NW], in_=x16_d[:, n0 : n0 + NW])

                oh = ohpool.tile([P, CBG_max * W], f16, tag="oh")
                nc.vector.tensor_tensor(
                    out=oh[:, : CBg * W].rearrange("p (c m) -> p c m", m=W),
                    in0=dstrel_s[:, co0:co1].to_broadcast([P, CBg, W]),
                    in1=it16[:, : CBg * W].rearrange("p (c m) -> p c m", m=W),
                    op=mybir.AluOpType.is_equal,
                )

                pm = pbin.tile([P, NW_max], f32, tag="pm")
                for w in range(w0, w1):
                    off, cb = wplan[w]
                    sw = w - w0
                    for i in range(cb):
                        lc = off - co0 + i
                        nc.tensor.matmul(
                            out=pm[:, sw * W : (sw + 1) * W],
                            lhsT=at8[:, lc * D : (lc + 1) * D],
                            rhs=oh[:, lc * W : (lc + 1) * W],
                            start=(i == 0),
                            stop=(i == cb - 1),
                        )
                mean_g = actpool.tile([P, NW_max], f16, tag="mean_g")
                nc.scalar.copy(out=mean_g[:, :NW], in_=pm[:, :NW])

                # --- MLP over this group (feature-major) ---
                ph1a = pmlp.tile([P, NW_max], f32, tag="h1a")
                ph1b = pmlp.tile([P, NW_max], f32, tag="h1b")
                nc.tensor.matmul(out=ph1a[:, :NW], lhsT=w1s_0[:, 0:P],
                                 rhs=xg[:, :NW], start=True, stop=False)
                nc.tensor.matmul(out=ph1a[:, :NW], lhsT=w1s_1[:, 0:P],
                                 rhs=mean_g[:, :NW], start=False, stop=True)
                nc.tensor.matmul(out=ph1b[:, :NW], lhsT=w1s_0[:, P:HIDDEN],
                                 rhs=xg[:, :NW], start=True, stop=False)
                nc.tensor.matmul(out=ph1b[:, :NW], lhsT=w1s_1[:, P:HIDDEN],
                                 rhs=mean_g[:, :NW], start=False, stop=True)
                h1a = actpool.tile([P, NW_max], f16, tag="h1a_s")
                h1b = actpool.tile([P, NW_max], f16, tag="h1b_s")
                nc.scalar.activation(out=h1a[:, :NW], in_=ph1a[:, :NW],
                                     func=Relu, bias=b1s_0[:, 0:1])
                nc.scalar.activation(out=h1b[:, :NW], in_=ph1b[:, :NW],
                                     func=Relu, bias=b1s_1[:, 0:1])

                ph2a = pmlp.tile([P, NW_max], f32, tag="h2a")
                ph2b = pmlp.tile([P, NW_max], f32, tag="h2b")
                nc.tensor.matmul(out=ph2a[:, :NW], lhsT=w2s_0[:, 0:P],
                                 rhs=h1a[:, :NW], start=True, stop=False)
                nc.tensor.matmul(out=ph2a[:, :NW], lhsT=w2s_1[:, 0:P],
                                 rhs=h1b[:, :NW], start=False, stop=True)
                nc.tensor.matmul(out=ph2b[:, :NW], lhsT=w2s_0[:, P:HIDDEN],
                                 rhs=h1a[:, :NW], start=True, stop=False)
                nc.tensor.matmul(out=ph2b[:, :NW], lhsT=w2s_1[:, P:HIDDEN],
                                 rhs=h1b[:, :NW], start=False, stop=True)
                h2a = actpool.tile([P, NW_max], f16, tag="h2a_s")
                h2b = actpool.tile([P, NW_max], f16, tag="h2b_s")
                nc.scalar.activation(out=h2a[:, :NW], in_=ph2a[:, :NW],
                                     func=Relu, bias=b2s_0[:, 0:1])
                nc.vector.tensor_scalar(
                    out=h2b[:, :NW], in0=ph2b[:, :NW],
                    scalar1=b2s_1[:, 0:1], scalar2=0.0,
                    op0=mybir.AluOpType.add, op1=mybir.AluOpType.max,
                )

                po = pmlp.tile([P, NW_max], f32, tag="h1a")
                nc.tensor.matmul(out=po[:, :NW], lhsT=w3s_0[:],
                                 rhs=h2a[:, :NW], start=True, stop=False)
                nc.tensor.matmul(out=po[:, :NW], lhsT=w3s_1[:],
                                 rhs=h2b[:, :NW], start=False, stop=True)
                og = actpool.tile([P, NW_max], f16, tag="og")
                nc.vector.tensor_scalar(
                    out=og[:, :NW], in0=po[:, :NW],
                    scalar1=b3s[:, 0:1], scalar2=None,
                    op0=mybir.AluOpType.add,
                )
                nc.sync.dma_start(
                    out=outT_d[:, n0 : n0 + NW], in_=og[:, :NW]
                )

    nc.finalize()
    return nc


def _pack_windows(profile):
    """DP over node ranks: choose window boundaries (<=W nodes each) to
    minimize chunk+window cost. profile = descending per-rank edge-count
    upper bound. Returns list of per-window node spans (r0, r1)."""
    n = len(profile)
    csum = np.zeros(n + 1, np.float64)
    csum[1:] = np.cumsum(profile)
    INF = float("inf")
    dp = np.full(n + 1, INF)
    prev = np.zeros(n + 1, np.int32)
    dp[0] = 0.0
    for r1 in range(1, n + 1):
        best = INF
        barg = r1 - 1
        for r0 in range(max(0, r1 - W), r1):
            if dp[r0] == INF:
                continue
            e = csum[r1] - csum[r0]
            cost = dp[r0] + CHUNK_COST * max((e + 127) // 128, 1) + WINDOW_COST
            if cost < best:
                best = cost
                barg = r0
        dp[r1] = best
        prev[r1] = barg
    spans = []
    r = n
    while r > 0:
        spans.append((int(prev[r]), r))
        r = int(prev[r])
    spans.reverse()
    return spans


def _host_prep(x, edge_index, edge_attr):
    """Sort/scale/pad edges; returns (META, per-core input arrays,
    per-core slot->global-node maps)."""
    col = np.asarray(edge_index)[1].astype(np.int64)
    x = np.asarray(x, dtype=np.float32)
    counts = np.bincount(col, minlength=N_NODES).astype(np.int64)
    scale = (1.0 / np.maximum(counts, 1)).astype(np.float32)

    eorder = np.argsort(col, kind="stable")
    col_s = col[eorder]
    attr_s = np.asarray(edge_attr, dtype=np.float32)[eorder]
    attr_s = attr_s * scale[col_s][:, None]
    attr_s8 = attr_s.astype(ml_dtypes.float8_e4m3)
    # edge start offset of each node in the dst-sorted arrays
    estart = np.zeros(N_NODES + 1, np.int64)
    estart[1:] = np.cumsum(counts)

    # per-core descending-count node order + rank-wise max profile
    nodeperm = np.empty((N_CORES, NPC_REAL), np.int64)   # rank -> local node
    cnt_sorted = np.empty((N_CORES, NPC_REAL), np.int64)
    for c in range(N_CORES):
        cnt_c = counts[c * NPC_REAL : (c + 1) * NPC_REAL]
        p = np.argsort(-cnt_c, kind="stable")
        nodeperm[c] = p
        cnt_sorted[c] = cnt_c[p]
    # plan on the rank-wise mean profile; per-core greedy fill spills
    # overflow forward, spare windows at the end absorb the tail
    profile = cnt_sorted.mean(axis=0)

    spans = _pack_windows(profile)
    n_windows = len(spans)
    ncols = np.array(
        [max(int(np.ceil(profile[r0:r1].sum() / 128)), 1) for r0, r1 in spans],
        np.int64,
    )
    n_spare = 24
    spans = spans + [(NPC_REAL, NPC_REAL)] * n_spare
    ncols = np.concatenate([ncols, np.ones(n_spare, np.int64)])
    n_windows += n_spare
    col_off = np.zeros(n_windows, np.int64)
    col_off[1:] = np.cumsum(ncols)[:-1]
    NCH = int(ncols.sum())
    E_pad = NCH * P
    NPC = n_windows * W

    gsizes = _group_plan(n_windows)
    gbounds = []
    w = 0
    for gs in gsizes:
        gbounds.append((w, w + gs))
        w += gs
    wplan = tuple((int(col_off[j]), int(ncols[j])) for j in range(n_windows))
    META = (wplan, tuple(gbounds))

    per_core = []
    slot_node = []  # per core: slot -> global node id (-1 empty)
    for c in range(N_CORES):
        cnts = cnt_sorted[c]
        # greedy fill: assign ranks to windows within the shared plan.
        # profile >= per-core counts rank-wise, so every core fits.
        win_of_rank = np.empty(NPC_REAL, np.int64)
        idx_of_rank = np.empty(NPC_REAL, np.int64)
        j = 0
        e_used = 0
        slots = 0
        for r in range(NPC_REAL):
            cap = ncols[j] * P
            while slots >= W or e_used + cnts[r] > cap:
                j += 1
                assert j < n_windows, "core overflowed the shared window plan"
                cap = ncols[j] * P
                e_used = 0
                slots = 0
            win_of_rank[r] = j
            idx_of_rank[r] = slots
            e_used += cnts[r]
            slots += 1

        # per-rank edge placement base: window chunk base + prefix of
        # earlier ranks in the same window
        wbase = col_off[win_of_rank] * P
        newwin = np.ones(NPC_REAL, bool)
        newwin[1:] = win_of_rank[1:] != win_of_rank[:-1]
        pre = np.cumsum(cnts) - cnts
        wstart = np.maximum.accumulate(np.where(newwin, pre, -1))
        prefix_in_win = pre - wstart
        rank_base = wbase + prefix_in_win

        # edges in rank order
        lnode = nodeperm[c]                    # rank -> local node
        gnode = lnode + c * NPC_REAL
        src0 = estart[gnode]                   # per-rank first edge (sorted)
        total = int(cnts.sum())
        src_idx = np.concatenate(
            [np.arange(src0[r], src0[r] + cnts[r]) for r in range(NPC_REAL)]
        ) if total else np.empty(0, np.int64)
        within = np.arange(total) - np.repeat(pre, cnts)
        edest = np.repeat(rank_base, cnts) + within

        attr_pad = np.zeros((E_pad, D), ml_dtypes.float8_e4m3)
        attr_pad[edest] = attr_s8[src_idx]
        attr8 = np.ascontiguousarray(
            attr_pad.reshape(NCH, P, D).transpose(1, 0, 2).reshape(P, NCH * D)
        )

        dstrel = np.full((E_pad,), 200.0, np.float16)
        dstrel[edest] = np.repeat(idx_of_rank, cnts).astype(np.float16)
        dstrelT = np.ascontiguousarray(dstrel.reshape(NCH, P).T)

        # node features + slot map
        slot = win_of_rank * W + idx_of_rank
        smap = np.full(NPC, -1, np.int64)
        smap[slot] = gnode
        xT = np.zeros((NPC, D), np.float16)
        xT[slot] = x[gnode].astype(np.float16)
        xT = np.ascontiguousarray(xT.T)

        per_core.append({"attr8": attr8, "dstrelT": dstrelT, "x16": xT})
        slot_node.append(smap)
    return META, per_core, slot_node


def _build_consts(b1, b2, b3):
    consts = np.zeros((P, 5), np.float32)
    consts[:, 0] = b1[:P]
    consts[:, 1] = b1[P:]
    consts[:, 2] = b2[:P]
    consts[:, 3] = b2[P:]
    consts[:, 4] = b3
    return consts


def _build_wts(W1, W2, W3):
    wts = np.empty((P, 4 * HIDDEN + 2 * DOUT), np.float16)
    wts[:, 0:HIDDEN] = W1[:P]
    wts[:, HIDDEN : 2 * HIDDEN] = W1[P:]
    wts[:, 2 * HIDDEN : 3 * HIDDEN] = W2[:P]
    wts[:, 3 * HIDDEN : 4 * HIDDEN] = W2[P:]
    wts[:, 4 * HIDDEN : 4 * HIDDEN + DOUT] = W3[:P]
    wts[:, 4 * HIDDEN + DOUT : 4 * HIDDEN + 2 * DOUT] = W3[P:]
    return wts


def _build_c16(META, dstrelT):
    """fp16 consts row-block: iota ramp | dstrel."""
    wplan, gbounds = META
    NCH = wplan[-1][0] + wplan[-1][1]
    CBG_max = max(
        wplan[w1 - 1][0] + wplan[w1 - 1][1] - wplan[w0][0] for w0, w1 in gbounds
    )
    c16 = np.empty((P, CBG_max * W + NCH), np.float16)
    c16[:, 0 : CBG_max * W] = np.tile(
        np.arange(W, dtype=np.float16), CBG_max
    )[None, :]
    c16[:, CBG_max * W :] = dstrelT
    return c16


def _make_in_maps(META, per_core, b1, b2, b3, W1, W2, W3):
    consts = _build_consts(b1, b2, b3)
    wts = _build_wts(W1, W2, W3)
    return [
        {
            "attr8": pc["attr8"].view(np.uint8),
            "x16": pc["x16"],
            "c16": _build_c16(META, pc["dstrelT"]),
            "consts": consts,
            "wts": wts,
        }
        for pc in per_core
    ]


def kernel(x, edge_index, edge_attr, W1, b1, W2, b2, W3, b3):
    META, per_core, slot_node = _host_prep(x, edge_index, edge_attr)

    if META not in _prog_cache:
        _prog_cache[META] = _build_program(META)
    nc = _prog_cache[META]

    in_maps = _make_in_maps(
        META, per_core,
        np.asarray(b1, np.float32), np.asarray(b2, np.float32),
        np.asarray(b3, np.float32),
        np.asarray(W1, np.float32), np.asarray(W2, np.float32),
        np.asarray(W3, np.float32),
    )
    res = run_bass_kernel_spmd(nc, in_maps, core_ids=list(range(N_CORES)))

    out = np.empty((N_NODES, DOUT), np.float32)
    for c in range(N_CORES):
        o = res.results[c]["outT"].T.astype(np.float32)
        smap = slot_node[c]
        m = smap >= 0
        out[smap[m]] = o[m]
    return out


# revision 36
# speedup vs baseline: 1.5219x; 1.1924x over previous
"""NodeNet GNN message-passing kernel for 8 Trainium2 NeuronCores.

Strategy (per sharding hint): shard nodes across the 8 cores; partition
edges by destination node on the host so the scatter-mean is device-local.

v2 — fp8 edge stream + padding-minimizing window packing:
  - Host sorts each core's 12,500 nodes by descending edge count and sorts
    edges by destination; edge rows are pre-scaled by 1/count(dst) so the
    device segment-sum directly yields the mean, then cast to fp8 e4m3
    (absmax error ~9e-3 vs the 2e-2 gate; the scatter-mean averages the
    quantization noise before the MLP sees it).
  - Windows hold up to W=16 nodes but close early so their edge lists land
    near 128-edge chunk boundaries (host DP over node ranks); the chunk
    plan is shared across cores (SPMD) and built from the rank-wise max
    count profile, so every core's greedy fill fits the plan.
  - Device builds, per group of windows, ONE batched [128 edge, chunk, W]
    fp16 one-hot (is_equal of dst-rel against an iota ramp, VectorE) and
    contracts chunk-by-chunk on the TensorEngine into per-window column
    slices of ONE per-group PSUM bank tile (fp8 lhsT x fp16 rhs, fp32
    accumulate) — one ScalarE evacuation per group instead of per window.
  - The 3-layer MLP runs feature-major per group (fp16 matmuls, fp32 PSUM)
    with PSUM evacuations split between ScalarE (mean, h1a, h1b, h2a) and
    VectorE (h2b, out) to keep both under the DMA roofline.

Cost-model timeline (per core): ~95 us DMA floor (~34 MB/core at 360
GB/s), PE ~67 us, ACT ~64 us, DVE ~61 us.
"""

import numpy as np
import ml_dtypes

import concourse.bacc as bacc
import concourse.mybir as mybir
import concourse.tile as tile
from concourse.bass_utils import run_bass_kernel_spmd

P = 128                    # partitions / matmul contraction tile
D = 128                    # node & edge feature dim
HIDDEN = 256
DOUT = 128
N_NODES = 100000
N_CORES = 8
NPC_REAL = 12500           # real nodes per core
W = 16                     # node slots per window (one-hot width)

# marginal cost weights for the host packing DP (ns, from the TRN2 cost
# model): one 128-edge chunk costs DMA 45.5 + PE 6.7 + DVE 16.7; one
# window costs 16 node slots of MLP/DMA work
CHUNK_COST = 70.0
WINDOW_COST = 176.0

_prog_cache: dict = {}

# engine assignment for the six PSUM evacuations (chain cadence tuning)
EVAC = {"mean": "V", "h1a": "A", "h1b": "V", "h2a": "A", "h2b": "V", "og": "A"}

f32 = mybir.dt.float32
f16 = mybir.dt.float16
f8 = mybir.dt.float8e4

Relu = mybir.ActivationFunctionType.Relu
Ident = mybir.ActivationFunctionType.Identity


def _evac(nc, key, out, in_, bias, relu=True):
    """PSUM->SBUF evacuation with bias (+relu) on ACT or DVE per EVAC."""
    if EVAC[key] == "A":
        if bias is None:
            nc.scalar.copy(out=out, in_=in_)
        else:
            nc.scalar.activation(out=out, in_=in_,
                                 func=(Relu if relu else Ident), bias=bias)
    else:
        if bias is None:
            nc.vector.tensor_copy(out=out, in_=in_)
        elif relu:
            nc.vector.tensor_scalar(
                out=out, in0=in_, scalar1=bias, scalar2=0.0,
                op0=mybir.AluOpType.add, op1=mybir.AluOpType.max,
            )
        else:
            nc.vector.tensor_scalar(
                out=out, in0=in_, scalar1=bias, scalar2=None,
                op0=mybir.AluOpType.add,
            )


def _group_plan(n_windows):
    """Group sizes in windows: small groups first (compute starts early),
    steady-state 32-window groups (512 node slots), aggressively tapered
    tail (the backlog drains at per-group chain latency, so the last
    chains must be short)."""
    # groups are PROCESSED in reversed plan order; both ends taper (fast
    # pipeline fill at the processing start, short chains at the drain)
    tail = [24, 16, 8]
    gsizes = [8, 16, 24]
    rem = n_windows - sum(gsizes) - sum(tail)
    while rem >= 32:
        gsizes.append(32)
        rem -= 32
    if rem:
        gsizes.append(rem)
    return gsizes + tail


def _build_program(META):
    """META = (wplan, gbounds) with wplan a tuple of (chunk_off, ncols)
    per window and gbounds a tuple of (w0, w1) per group; identical
    across cores (SPMD)."""
    wplan, gbounds = META
    n_windows = len(wplan)
    NPC = n_windows * W
    NCH = wplan[-1][0] + wplan[-1][1]
    CBG_max = max(
        wplan[w1 - 1][0] + wplan[w1 - 1][1] - wplan[w0][0] for w0, w1 in gbounds
    )
    NW_max = max(w1 - w0 for w0, w1 in gbounds) * W

    nc = bacc.Bacc(None)
    attr8_d = nc.dram_tensor("attr8", [P, NCH * D], f8, kind="ExternalInput")
    x16_d = nc.dram_tensor("x16", [P, NPC], f16, kind="ExternalInput")
    # fp16 consts: iota ramp (CBG_max*W) | dstrel (NCH)
    c16_d = nc.dram_tensor("c16", [P, CBG_max * W + NCH], f16, kind="ExternalInput")
    consts_d = nc.dram_tensor("consts", [P, 5], f32, kind="ExternalInput")
    wts_d = nc.dram_tensor("wts", [P, 4 * HIDDEN + 2 * DOUT], f16,
                           kind="ExternalInput")
    outT_d = nc.dram_tensor("outT", [P, NPC], f16, kind="ExternalOutput")

    with tile.TileContext(nc) as tc:
        with (
            tc.tile_pool(name="const", bufs=1) as cpool,
            tc.tile_pool(name="attr", bufs=8) as apool,
            tc.tile_pool(name="x", bufs=4) as xpool,
            tc.tile_pool(name="oh", bufs=5) as ohpool,
            tc.tile_pool(name="acts", bufs=3) as actpool,
            tc.tile_pool(name="pbin", bufs=2, space="PSUM") as pbin,
            tc.tile_pool(name="pmlp", bufs=1, space="PSUM") as pmlp,
            tc.tile_pool(name="ppo", bufs=2, space="PSUM") as ppo,
        ):
            cs = cpool.tile([P, 5], f32, tag="consts")
            ws = cpool.tile([P, 4 * HIDDEN + 2 * DOUT], f16, tag="wts")
            c16 = cpool.tile([P, CBG_max * W + NCH], f16, tag="c16")
            w1s_0 = ws[:, 0:HIDDEN]
            w1s_1 = ws[:, HIDDEN : 2 * HIDDEN]
            w2s_0 = ws[:, 2 * HIDDEN : 3 * HIDDEN]
            w2s_1 = ws[:, 3 * HIDDEN : 4 * HIDDEN]
            w3s_0 = ws[:, 4 * HIDDEN : 4 * HIDDEN + DOUT]
            w3s_1 = ws[:, 4 * HIDDEN + DOUT : 4 * HIDDEN + 2 * DOUT]
            b1s_0 = cs[:, 0:1]
            b1s_1 = cs[:, 1:2]
            b2s_0 = cs[:, 2:3]
            b2s_1 = cs[:, 3:4]
            b3s = cs[:, 4:5]
            it16 = c16[:, 0 : CBG_max * W]
            dstrel_s = c16[:, CBG_max * W : CBG_max * W + NCH]
            oall = cpool.tile([P, NPC], f16, tag="oall")

            # deferred slab stores: flush finished output columns with a
            # 2-group lag so the store never races its own evacuations
            SLAB = 4
            LAG = 2
            flushed = 0

            def build_oh(gi):
                """One-hot for group gi — depends only on c16, so it is
                emitted one group ahead: the in-order DVE queue must not
                park it behind the previous group's MLP evacuations."""
                bw0, bw1 = gbounds[gi]
                bco0 = wplan[bw0][0]
                bco1 = wplan[bw1 - 1][0] + wplan[bw1 - 1][1]
                bCB = bco1 - bco0
                oh = ohpool.tile([P, CBG_max * W], f16, tag="oh")
                nc.vector.tensor_tensor(
                    out=oh[:, : bCB * W].rearrange("p (c m) -> p c m", m=W),
                    in0=dstrel_s[:, bco0:bco1].to_broadcast([P, bCB, W]),
                    in1=it16[:, : bCB * W].rearrange("p (c m) -> p c m", m=W),
                    op=mybir.AluOpType.is_equal,
                )
                return oh

            # process groups smallest-first (reversed plan order): the big
            # groups' delivery cadence exceeds their compute cadence, so the
            # compute backlog shrinks toward the end and the drain collapses
            # to roughly one group's chain latency
            porder = list(range(len(gbounds)))[::-1]
            oh_next = None
            flush_hi = NPC
            for gi, g in enumerate(porder):
                w0, w1 = gbounds[g]
                NW = (w1 - w0) * W
                n0 = w0 * W
                co0 = wplan[w0][0]
                co1 = wplan[w1 - 1][0] + wplan[w1 - 1][1]
                CBg = co1 - co0

                at8 = apool.tile([P, CBG_max * D], f8, tag="attr")
                nc.sync.dma_start(
                    out=at8[:, : CBg * D], in_=attr8_d[:, co0 * D : co1 * D]
                )
                if gi == 0:
                    nc.sync.dma_start(out=c16[:], in_=c16_d[:, :])
                    nc.sync.dma_start(out=cs[:], in_=consts_d[:, :])
                    nc.sync.dma_start(out=ws[:], in_=wts_d[:, :])
                xg = xpool.tile([P, NW_max], f16, tag="x")
                nc.sync.dma_start(out=xg[:, :NW], in_=x16_d[:, n0 : n0 + NW])
                if gi >= SLAB + LAG and gi % SLAB == 0:
                    # issued from the (idle) Pool engine: a store waiting on
                    # its evacuations cannot head-of-line block the SP queue
                    f0 = gbounds[porder[gi - LAG]][0] * W
                    nc.gpsimd.dma_start(
                        out=outT_d[:, f0:flush_hi], in_=oall[:, f0:flush_hi]
                    )
                    flush_hi = f0

                oh = oh_next if oh_next is not None else build_oh(g)
                if gi + 1 < len(porder):
                    oh_next = build_oh(porder[gi + 1])

                pm = pbin.tile([P, NW_max], f32, tag="pm")
                for w in range(w0, w1):
                    off, cb = wplan[w]
                    sw = w - w0
                    for i in range(cb):
                        lc = off - co0 + i
                        nc.tensor.matmul(
                            out=pm[:, sw * W : (sw + 1) * W],
                            lhsT=at8[:, lc * D : (lc + 1) * D],
                            rhs=oh[:, lc * W : (lc + 1) * W],
                            start=(i == 0),
                            stop=(i == cb - 1),
                        )
                mean_g = actpool.tile([P, NW_max], f16, tag="mean_g")
                _evac(nc, "mean", mean_g[:, :NW], pm[:, :NW], None)

                # --- MLP over this group (feature-major) ---
                ph1a = pmlp.tile([P, NW_max], f32, tag="h1a")
                ph1b = pmlp.tile([P, NW_max], f32, tag="h1b")
                nc.tensor.matmul(out=ph1a[:, :NW], lhsT=w1s_0[:, 0:P],
                                 rhs=xg[:, :NW], start=True, stop=False)
                nc.tensor.matmul(out=ph1a[:, :NW], lhsT=w1s_1[:, 0:P],
                                 rhs=mean_g[:, :NW], start=False, stop=True)
                nc.tensor.matmul(out=ph1b[:, :NW], lhsT=w1s_0[:, P:HIDDEN],
                                 rhs=xg[:, :NW], start=True, stop=False)
                nc.tensor.matmul(out=ph1b[:, :NW], lhsT=w1s_1[:, P:HIDDEN],
                                 rhs=mean_g[:, :NW], start=False, stop=True)
                h1a = actpool.tile([P, NW_max], f16, tag="h1a_s")
                h1b = actpool.tile([P, NW_max], f16, tag="h1b_s")
                # paired evacuations on different engines run in parallel
                # instead of back-to-back on one in-order queue
                _evac(nc, "h1a", h1a[:, :NW], ph1a[:, :NW], b1s_0[:, 0:1])
                _evac(nc, "h1b", h1b[:, :NW], ph1b[:, :NW], b1s_1[:, 0:1])

                ph2a = pmlp.tile([P, NW_max], f32, tag="h2a")
                ph2b = pmlp.tile([P, NW_max], f32, tag="h2b")
                nc.tensor.matmul(out=ph2a[:, :NW], lhsT=w2s_0[:, 0:P],
                                 rhs=h1a[:, :NW], start=True, stop=False)
                nc.tensor.matmul(out=ph2a[:, :NW], lhsT=w2s_1[:, 0:P],
                                 rhs=h1b[:, :NW], start=False, stop=True)
                nc.tensor.matmul(out=ph2b[:, :NW], lhsT=w2s_0[:, P:HIDDEN],
                                 rhs=h1a[:, :NW], start=True, stop=False)
                nc.tensor.matmul(out=ph2b[:, :NW], lhsT=w2s_1[:, P:HIDDEN],
                                 rhs=h1b[:, :NW], start=False, stop=True)
                h2a = actpool.tile([P, NW_max], f16, tag="h2a_s")
                h2b = actpool.tile([P, NW_max], f16, tag="h2b_s")
                _evac(nc, "h2a", h2a[:, :NW], ph2a[:, :NW], b2s_0[:, 0:1])
                _evac(nc, "h2b", h2b[:, :NW], ph2b[:, :NW], b2s_1[:, 0:1])

                po = ppo.tile([P, NW_max], f32, tag="po")
                nc.tensor.matmul(out=po[:, :NW], lhsT=w3s_0[:],
                                 rhs=h2a[:, :NW], start=True, stop=False)
                nc.tensor.matmul(out=po[:, :NW], lhsT=w3s_1[:],
                                 rhs=h2b[:, :NW], start=False, stop=True)
                _evac(nc, "og", oall[:, n0 : n0 + NW], po[:, :NW],
                      b3s[:, 0:1], relu=False)

            nc.gpsimd.dma_start(
                out=outT_d[:, 0:flush_hi], in_=oall[:, 0:flush_hi]
            )

    nc.finalize()
    return nc


def _pack_windows(profile):
    """DP over node ranks: choose window boundaries (<=W nodes each) to
    minimize chunk+window cost. profile = descending per-rank edge-count
    upper bound. Returns list of per-window node spans (r0, r1)."""
    n = len(profile)
    csum = np.zeros(n + 1, np.float64)
    csum[1:] = np.cumsum(profile)
    INF = float("inf")
    dp = np.full(n + 1, INF)
    prev = np.zeros(n + 1, np.int32)
    dp[0] = 0.0
    for r1 in range(1, n + 1):
        best = INF
        barg = r1 - 1
        for r0 in range(max(0, r1 - W), r1):
            if dp[r0] == INF:
                continue
            e = csum[r1] - csum[r0]
            cost = dp[r0] + CHUNK_COST * max((e + 127) // 128, 1) + WINDOW_COST
            if cost < best:
                best = cost
                barg = r0
        dp[r1] = best
        prev[r1] = barg
    spans = []
    r = n
    while r > 0:
        spans.append((int(prev[r]), r))
        r = int(prev[r])
    spans.reverse()
    return spans


def _host_prep(x, edge_index, edge_attr):
    """Sort/scale/pad edges; returns (META, per-core input arrays,
    per-core slot->global-node maps)."""
    col = np.asarray(edge_index)[1].astype(np.int64)
    x = np.asarray(x, dtype=np.float32)
    counts = np.bincount(col, minlength=N_NODES).astype(np.int64)
    scale = (1.0 / np.maximum(counts, 1)).astype(np.float32)

    eorder = np.argsort(col, kind="stable")
    col_s = col[eorder]
    attr_s = np.asarray(edge_attr, dtype=np.float32)[eorder]
    attr_s = attr_s * scale[col_s][:, None]
    attr_s8 = attr_s.astype(ml_dtypes.float8_e4m3)
    # edge start offset of each node in the dst-sorted arrays
    estart = np.zeros(N_NODES + 1, np.int64)
    estart[1:] = np.cumsum(counts)

    # per-core descending-count node order + rank-wise max profile
    nodeperm = np.empty((N_CORES, NPC_REAL), np.int64)   # rank -> local node
    cnt_sorted = np.empty((N_CORES, NPC_REAL), np.int64)
    for c in range(N_CORES):
        cnt_c = counts[c * NPC_REAL : (c + 1) * NPC_REAL]
        p = np.argsort(-cnt_c, kind="stable")
        nodeperm[c] = p
        cnt_sorted[c] = cnt_c[p]
    # plan on the rank-wise mean profile; per-core greedy fill spills
    # overflow forward, spare windows at the end absorb the tail
    profile = cnt_sorted.mean(axis=0)

    spans = _pack_windows(profile)
    n_windows = len(spans)
    ncols = np.array(
        [max(int(np.ceil(profile[r0:r1].sum() / 128)), 1) for r0, r1 in spans],
        np.int64,
    )
    n_spare = 24
    spans = spans + [(NPC_REAL, NPC_REAL)] * n_spare
    ncols = np.concatenate([ncols, np.ones(n_spare, np.int64)])
    n_windows += n_spare
    col_off = np.zeros(n_windows, np.int64)
    col_off[1:] = np.cumsum(ncols)[:-1]
    NCH = int(ncols.sum())
    E_pad = NCH * P
    NPC = n_windows * W

    gsizes = _group_plan(n_windows)
    gbounds = []
    w = 0
    for gs in gsizes:
        gbounds.append((w, w + gs))
        w += gs
    wplan = tuple((int(col_off[j]), int(ncols[j])) for j in range(n_windows))
    META = (wplan, tuple(gbounds))

    per_core = []
    slot_node = []  # per core: slot -> global node id (-1 empty)
    for c in range(N_CORES):
        cnts = cnt_sorted[c]
        # greedy fill: assign ranks to windows within the shared plan.
        # profile >= per-core counts rank-wise, so every core fits.
        win_of_rank = np.empty(NPC_REAL, np.int64)
        idx_of_rank = np.empty(NPC_REAL, np.int64)
        j = 0
        e_used = 0
        slots = 0
        for r in range(NPC_REAL):
            cap = ncols[j] * P
            while slots >= W or e_used + cnts[r] > cap:
                j += 1
                assert j < n_windows, "core overflowed the shared window plan"
                cap = ncols[j] * P
                e_used = 0
                slots = 0
            win_of_rank[r] = j
            idx_of_rank[r] = slots
            e_used += cnts[r]
            slots += 1

        # per-rank edge placement base: window chunk base + prefix of
        # earlier ranks in the same window
        wbase = col_off[win_of_rank] * P
        newwin = np.ones(NPC_REAL, bool)
        newwin[1:] = win_of_rank[1:] != win_of_rank[:-1]
        pre = np.cumsum(cnts) - cnts
        wstart = np.maximum.accumulate(np.where(newwin, pre, -1))
        prefix_in_win = pre - wstart
        rank_base = wbase + prefix_in_win

        # edges in rank order
        lnode = nodeperm[c]                    # rank -> local node
        gnode = lnode + c * NPC_REAL
        src0 = estart[gnode]                   # per-rank first edge (sorted)
        total = int(cnts.sum())
        src_idx = np.concatenate(
            [np.arange(src0[r], src0[r] + cnts[r]) for r in range(NPC_REAL)]
        ) if total else np.empty(0, np.int64)
        within = np.arange(total) - np.repeat(pre, cnts)
        edest = np.repeat(rank_base, cnts) + within

        attr_pad = np.zeros((E_pad, D), ml_dtypes.float8_e4m3)
        attr_pad[edest] = attr_s8[src_idx]
        attr8 = np.ascontiguousarray(
            attr_pad.reshape(NCH, P, D).transpose(1, 0, 2).reshape(P, NCH * D)
        )

        dstrel = np.full((E_pad,), 200.0, np.float16)
        dstrel[edest] = np.repeat(idx_of_rank, cnts).astype(np.float16)
        dstrelT = np.ascontiguousarray(dstrel.reshape(NCH, P).T)

        # node features + slot map
        slot = win_of_rank * W + idx_of_rank
        smap = np.full(NPC, -1, np.int64)
        smap[slot] = gnode
        xT = np.zeros((NPC, D), np.float16)
        xT[slot] = x[gnode].astype(np.float16)
        xT = np.ascontiguousarray(xT.T)

        per_core.append({"attr8": attr8, "dstrelT": dstrelT, "x16": xT})
        slot_node.append(smap)
    return META, per_core, slot_node


def _build_consts(b1, b2, b3):
    consts = np.zeros((P, 5), np.float32)
    consts[:, 0] = b1[:P]
    consts[:, 1] = b1[P:]
    consts[:, 2] = b2[:P]
    consts[:, 3] = b2[P:]
    consts[:, 4] = b3
    return consts


def _build_wts(W1, W2, W3):
    wts = np.empty((P, 4 * HIDDEN + 2 * DOUT), np.float16)
    wts[:, 0:HIDDEN] = W1[:P]
    wts[:, HIDDEN : 2 * HIDDEN] = W1[P:]
    wts[:, 2 * HIDDEN : 3 * HIDDEN] = W2[:P]
    wts[:, 3 * HIDDEN : 4 * HIDDEN] = W2[P:]
    wts[:, 4 * HIDDEN : 4 * HIDDEN + DOUT] = W3[:P]
    wts[:, 4 * HIDDEN + DOUT : 4 * HIDDEN + 2 * DOUT] = W3[P:]
    return wts


def _build_c16(META, dstrelT):
    """fp16 consts row-block: iota ramp | dstrel."""
    wplan, gbounds = META
    NCH = wplan[-1][0] + wplan[-1][1]
    CBG_max = max(
        wplan[w1 - 1][0] + wplan[w1 - 1][1] - wplan[w0][0] for w0, w1 in gbounds
    )
    c16 = np.empty((P, CBG_max * W + NCH), np.float16)
    c16[:, 0 : CBG_max * W] = np.tile(
        np.arange(W, dtype=np.float16), CBG_max
    )[None, :]
    c16[:, CBG_max * W :] = dstrelT
    return c16


def _make_in_maps(META, per_core, b1, b2, b3, W1, W2, W3):
    consts = _build_consts(b1, b2, b3)
    wts = _build_wts(W1, W2, W3)
    return [
        {
            "attr8": pc["attr8"].view(np.uint8),
            "x16": pc["x16"],
            "c16": _build_c16(META, pc["dstrelT"]),
            "consts": consts,
            "wts": wts,
        }
        for pc in per_core
    ]


def kernel(x, edge_index, edge_attr, W1, b1, W2, b2, W3, b3):
    META, per_core, slot_node = _host_prep(x, edge_index, edge_attr)

    if META not in _prog_cache:
        _prog_cache[META] = _build_program(META)
    nc = _prog_cache[META]

    in_maps = _make_in_maps(
        META, per_core,
        np.asarray(b1, np.float32), np.asarray(b2, np.float32),
        np.asarray(b3, np.float32),
        np.asarray(W1, np.float32), np.asarray(W2, np.float32),
        np.asarray(W3, np.float32),
    )
    res = run_bass_kernel_spmd(nc, in_maps, core_ids=list(range(N_CORES)))

    out = np.empty((N_NODES, DOUT), np.float32)
    for c in range(N_CORES):
        o = res.results[c]["outT"].T.astype(np.float32)
        smap = slot_node[c]
        m = smap >= 0
        out[smap[m]] = o[m]
    return out


# revision 82
# speedup vs baseline: 1.6806x; 1.1042x over previous
"""NodeNet GNN message-passing kernel for 8 Trainium2 NeuronCores.

Strategy (per sharding hint): shard nodes across the 8 cores; partition
edges by destination node on the host so the scatter-mean is device-local.

v2 — fp8 edge stream + padding-minimizing window packing:
  - Host sorts each core's 12,500 nodes by descending edge count and sorts
    edges by destination; edge rows are pre-scaled by 1/count(dst) so the
    device segment-sum directly yields the mean, then cast to fp8 e4m3
    (absmax error ~9e-3 vs the 2e-2 gate; the scatter-mean averages the
    quantization noise before the MLP sees it).
  - Windows hold up to W=16 nodes but close early so their edge lists land
    near 128-edge chunk boundaries (host DP over node ranks); the chunk
    plan is shared across cores (SPMD) and built from the rank-wise max
    count profile, so every core's greedy fill fits the plan.
  - Device builds, per group of windows, ONE batched [128 edge, chunk, W]
    fp16 one-hot (is_equal of dst-rel against an iota ramp, VectorE) and
    contracts chunk-by-chunk on the TensorEngine into per-window column
    slices of ONE per-group PSUM bank tile (fp8 lhsT x fp16 rhs, fp32
    accumulate) — one ScalarE evacuation per group instead of per window.
  - The 3-layer MLP runs feature-major per group (fp16 matmuls, fp32 PSUM)
    with PSUM evacuations split between ScalarE (mean, h1a, h1b, h2a) and
    VectorE (h2b, out) to keep both under the DMA roofline.

Cost-model timeline (per core): ~95 us DMA floor (~34 MB/core at 360
GB/s), PE ~67 us, ACT ~64 us, DVE ~61 us.
"""

import numpy as np
import ml_dtypes

import concourse.bacc as bacc
import concourse.mybir as mybir
import concourse.tile as tile
from concourse.bass_utils import run_bass_kernel_spmd

P = 128                    # partitions / matmul contraction tile
D = 128                    # node & edge feature dim
HIDDEN = 256
DOUT = 128
N_NODES = 100000
N_CORES = 8
NPC_REAL = 12500           # real nodes per core
W = 16                     # node slots per window (one-hot width)

# marginal cost weights for the host packing DP (ns, from the TRN2 cost
# model): one 128-edge chunk costs DMA 45.5 + PE 6.7 + DVE 16.7; one
# window costs 16 node slots of MLP/DMA work
CHUNK_COST = 70.0
WINDOW_COST = 90.0
PLAN_SLACK = 0     # slots per window the DP leaves for tail fillers

_prog_cache: dict = {}

# engine assignment for the six PSUM evacuations (chain cadence tuning)
EVAC = {"mean": "A", "h1a": "A", "h1b": "V", "h2a": "A", "h2b": "V", "og": "A"}
EXP_ENG = "A"     # dstrel expand: "A" ScalarE, "P" Pool, "N" none (1x)
STORE_ENG = "P"   # slab stores: "P" Pool SWDGE, "S" SP queue
STORE_LAG = 2     # groups of lag before a finished slab is stored
OH_PAIR = 1       # processed groups sharing one expand+is_equal pair


def _oh_plan(gbounds, gchunks):
    """Processing order (reversed plan) and one-hot batches: OH_PAIR
    consecutive processed groups share one contiguous chunk span."""
    porder = list(range(len(gbounds)))[::-1]
    batches = []
    for i in range(0, len(porder), OH_PAIR):
        mem = porder[i : i + OH_PAIR]
        base = min(gchunks[g][0] for g in mem)
        top = max(gchunks[g][1] for g in mem)
        batches.append((base, top, tuple(mem)))
    return porder, batches

f32 = mybir.dt.float32
f16 = mybir.dt.float16
f8 = mybir.dt.float8e4

Relu = mybir.ActivationFunctionType.Relu
Ident = mybir.ActivationFunctionType.Identity


def _evac(nc, key, out, in_, bias, relu=True):
    """PSUM->SBUF evacuation with bias (+relu) on ACT or DVE per EVAC."""
    if EVAC[key] == "A":
        if bias is None:
            nc.scalar.copy(out=out, in_=in_)
        else:
            nc.scalar.activation(out=out, in_=in_,
                                 func=(Relu if relu else Ident), bias=bias)
    else:
        if bias is None:
            nc.vector.tensor_copy(out=out, in_=in_)
        elif relu:
            nc.vector.tensor_scalar(
                out=out, in0=in_, scalar1=bias, scalar2=0.0,
                op0=mybir.AluOpType.add, op1=mybir.AluOpType.max,
            )
        else:
            nc.vector.tensor_scalar(
                out=out, in0=in_, scalar1=bias, scalar2=None,
                op0=mybir.AluOpType.add,
            )


def _group_plan(n_windows):
    """Group sizes in windows: small groups first (compute starts early),
    steady-state 32-window groups (512 node slots), aggressively tapered
    tail (the backlog drains at per-group chain latency, so the last
    chains must be short)."""
    # groups are PROCESSED in reversed plan order; both ends taper (fast
    # pipeline fill at the processing start, short chains at the drain)
    gpw = 512 // W  # windows per full 512-column group
    tail = [(3 * gpw) // 4, gpw // 2, gpw // 4]
    gsizes = list(tail[::-1])
    rem = n_windows - sum(gsizes) - sum(tail)
    while rem >= gpw:
        gsizes.append(gpw)
        rem -= gpw
    if rem:
        gsizes.append(rem)
    return gsizes + tail


def _build_program(META):
    """META = (gbounds, wparts, gchunks, NCH): per-group window bounds,
    per-window matmul parts (chunk col, partition lo, hi), per-group
    chunk ranges; identical across cores (SPMD)."""
    gbounds, wparts, gchunks, NCH = META
    n_windows = len(wparts)
    NPC = n_windows * W
    CBG_max = max(c1 - c0 for c0, c1 in gchunks)
    NW_max = max(w1 - w0 for w0, w1 in gbounds) * W
    porder, oh_batches = _oh_plan(gbounds, gchunks)
    CB_OH = max(top - base for base, top, _ in oh_batches)
    batch_of = {}
    for bi, (_, _, mem) in enumerate(oh_batches):
        for g in mem:
            batch_of[g] = bi

    nc = bacc.Bacc(None)
    attr8_d = nc.dram_tensor("attr8", [P, NCH * D], f8, kind="ExternalInput")
    x16_d = nc.dram_tensor("x16", [P, NPC], f16, kind="ExternalInput")
    # fp16 consts: iota ramp (CB_OH*W) | dstrel (NCH)
    c16_d = nc.dram_tensor("c16", [P, CB_OH * W + NCH], f16, kind="ExternalInput")
    consts_d = nc.dram_tensor("consts", [P, 5], f32, kind="ExternalInput")
    wts_d = nc.dram_tensor("wts", [P, 4 * HIDDEN + 2 * DOUT], f16,
                           kind="ExternalInput")
    outT_d = nc.dram_tensor("outT", [P, NPC], f16, kind="ExternalOutput")

    with tile.TileContext(nc) as tc:
        with (
            tc.tile_pool(name="const", bufs=1) as cpool,
            tc.tile_pool(name="attr", bufs=8) as apool,
            tc.tile_pool(name="x", bufs=4) as xpool,
            tc.tile_pool(name="oh", bufs=5) as ohpool,
            tc.tile_pool(name="dw", bufs=3) as dwpool,
            tc.tile_pool(name="acts", bufs=3) as actpool,
            tc.tile_pool(name="pbin", bufs=2, space="PSUM") as pbin,
            tc.tile_pool(name="pmlp", bufs=1, space="PSUM") as pmlp,
            tc.tile_pool(name="ppo", bufs=2, space="PSUM") as ppo,
        ):
            cs = cpool.tile([P, 5], f32, tag="consts")
            ws = cpool.tile([P, 4 * HIDDEN + 2 * DOUT], f16, tag="wts")
            c16 = cpool.tile([P, CB_OH * W + NCH], f16, tag="c16")
            w1s_0 = ws[:, 0:HIDDEN]
            w1s_1 = ws[:, HIDDEN : 2 * HIDDEN]
            w2s_0 = ws[:, 2 * HIDDEN : 3 * HIDDEN]
            w2s_1 = ws[:, 3 * HIDDEN : 4 * HIDDEN]
            w3s_0 = ws[:, 4 * HIDDEN : 4 * HIDDEN + DOUT]
            w3s_1 = ws[:, 4 * HIDDEN + DOUT : 4 * HIDDEN + 2 * DOUT]
            b1s_0 = cs[:, 0:1]
            b1s_1 = cs[:, 1:2]
            b2s_0 = cs[:, 2:3]
            b2s_1 = cs[:, 3:4]
            b3s = cs[:, 4:5]
            it16 = c16[:, 0 : CB_OH * W]
            dstrel_s = c16[:, CB_OH * W : CB_OH * W + NCH]
            oall = cpool.tile([P, NPC], f16, tag="oall")

            # deferred slab stores: flush finished output columns with a
            # lag so the store never races its own evacuations
            SLAB = 4
            LAG = STORE_LAG
            store_dma = (nc.gpsimd.dma_start if STORE_ENG == "P"
                         else nc.sync.dma_start)

            def build_oh(bi):
                """One-hot for batch bi (OH_PAIR processed groups' chunk
                span) — depends only on c16. ScalarE expands dstrel to a
                packed wide tile first: a broadcast (stride-0) operand
                would deny the DVE is_equal its 2x mode."""
                bco0, bco1, _ = oh_batches[bi]
                bCB = bco1 - bco0
                oh = ohpool.tile([P, CB_OH * W], f16, tag="oh")
                if EXP_ENG == "N":
                    # plain 1x is_equal with the broadcast operand (all DVE)
                    nc.vector.tensor_tensor(
                        out=oh[:, : bCB * W].rearrange("p (c m) -> p c m", m=W),
                        in0=dstrel_s[:, bco0:bco1].to_broadcast([P, bCB, W]),
                        in1=it16[:, : bCB * W].rearrange("p (c m) -> p c m", m=W),
                        op=mybir.AluOpType.is_equal,
                    )
                    return oh
                dw = dwpool.tile([P, CB_OH * W], f16, tag="dw")
                exp_copy = {"A": nc.scalar.copy, "P": nc.gpsimd.tensor_copy,
                            "V": nc.vector.tensor_copy}[EXP_ENG]
                exp_copy(
                    out=dw[:, : bCB * W].rearrange("p (c m) -> p c m", m=W),
                    in_=dstrel_s[:, bco0:bco1].to_broadcast([P, bCB, W]),
                )
                nc.vector.tensor_tensor(
                    out=oh[:, : bCB * W],
                    in0=dw[:, : bCB * W],
                    in1=it16[:, : bCB * W],
                    op=mybir.AluOpType.is_equal,
                )
                return oh

            # process groups smallest-first (reversed plan order): the big
            # groups' delivery cadence exceeds their compute cadence, so the
            # compute backlog shrinks toward the end and the drain collapses
            # to roughly one group's chain latency
            oh_cache = {}   # batch index -> oh tile
            flush_hi = NPC
            first = True
            for gi, g in enumerate(porder):
                w0, w1 = gbounds[g]
                NW = (w1 - w0) * W
                n0 = w0 * W
                co0, co1 = gchunks[g]
                CBg = co1 - co0
                if CBg == 0:
                    # no core placed any node here (shrunk spare windows)
                    continue

                at8 = apool.tile([P, CBG_max * D], f8, tag="attr")
                nc.sync.dma_start(
                    out=at8[:, : CBg * D], in_=attr8_d[:, co0 * D : co1 * D]
                )
                if first:
                    first = False
                    nc.sync.dma_start(out=c16[:], in_=c16_d[:, :])
                    nc.sync.dma_start(out=cs[:], in_=consts_d[:, :])
                    nc.sync.dma_start(out=ws[:], in_=wts_d[:, :])
                xg = xpool.tile([P, NW_max], f16, tag="x")
                nc.sync.dma_start(out=xg[:, :NW], in_=x16_d[:, n0 : n0 + NW])
                if gi >= SLAB + LAG and gi % SLAB == 0:
                    f0 = gbounds[porder[gi - LAG]][0] * W
                    store_dma(
                        out=outT_d[:, f0:flush_hi], in_=oall[:, f0:flush_hi]
                    )
                    flush_hi = f0

                bi = batch_of[g]
                if bi not in oh_cache:
                    oh_cache = {bi: build_oh(bi)}
                oh = oh_cache[bi]
                ob = oh_batches[bi][0]  # batch chunk base

                pm = pbin.tile([P, NW_max], f32, tag="pm")
                for w in range(w0, w1):
                    parts = wparts[w]
                    sw = w - w0
                    for i, (c, p0, p1) in enumerate(parts):
                        lc = c - co0
                        lo = c - ob
                        nc.tensor.matmul(
                            out=pm[:, sw * W : (sw + 1) * W],
                            lhsT=at8[p0:p1, lc * D : (lc + 1) * D],
                            rhs=oh[p0:p1, lo * W : (lo + 1) * W],
                            start=(i == 0),
                            stop=(i == len(parts) - 1),
                        )
                mean_g = actpool.tile([P, NW_max], f16, tag="mean_g")
                _evac(nc, "mean", mean_g[:, :NW], pm[:, :NW], None)

                # --- MLP over this group (feature-major) ---
                ph1a = pmlp.tile([P, NW_max], f32, tag="h1a")
                ph1b = pmlp.tile([P, NW_max], f32, tag="h1b")
                nc.tensor.matmul(out=ph1a[:, :NW], lhsT=w1s_0[:, 0:P],
                                 rhs=xg[:, :NW], start=True, stop=False)
                nc.tensor.matmul(out=ph1a[:, :NW], lhsT=w1s_1[:, 0:P],
                                 rhs=mean_g[:, :NW], start=False, stop=True)
                nc.tensor.matmul(out=ph1b[:, :NW], lhsT=w1s_0[:, P:HIDDEN],
                                 rhs=xg[:, :NW], start=True, stop=False)
                nc.tensor.matmul(out=ph1b[:, :NW], lhsT=w1s_1[:, P:HIDDEN],
                                 rhs=mean_g[:, :NW], start=False, stop=True)
                h1a = actpool.tile([P, NW_max], f16, tag="h1a_s")
                h1b = actpool.tile([P, NW_max], f16, tag="h1b_s")
                # paired evacuations on different engines run in parallel
                # instead of back-to-back on one in-order queue
                _evac(nc, "h1a", h1a[:, :NW], ph1a[:, :NW], b1s_0[:, 0:1])
                _evac(nc, "h1b", h1b[:, :NW], ph1b[:, :NW], b1s_1[:, 0:1])

                ph2a = pmlp.tile([P, NW_max], f32, tag="h2a")
                ph2b = pmlp.tile([P, NW_max], f32, tag="h2b")
                nc.tensor.matmul(out=ph2a[:, :NW], lhsT=w2s_0[:, 0:P],
                                 rhs=h1a[:, :NW], start=True, stop=False)
                nc.tensor.matmul(out=ph2a[:, :NW], lhsT=w2s_1[:, 0:P],
                                 rhs=h1b[:, :NW], start=False, stop=True)
                nc.tensor.matmul(out=ph2b[:, :NW], lhsT=w2s_0[:, P:HIDDEN],
                                 rhs=h1a[:, :NW], start=True, stop=False)
                nc.tensor.matmul(out=ph2b[:, :NW], lhsT=w2s_1[:, P:HIDDEN],
                                 rhs=h1b[:, :NW], start=False, stop=True)
                h2a = actpool.tile([P, NW_max], f16, tag="h2a_s")
                h2b = actpool.tile([P, NW_max], f16, tag="h2b_s")
                _evac(nc, "h2a", h2a[:, :NW], ph2a[:, :NW], b2s_0[:, 0:1])
                _evac(nc, "h2b", h2b[:, :NW], ph2b[:, :NW], b2s_1[:, 0:1])

                po = ppo.tile([P, NW_max], f32, tag="po")
                nc.tensor.matmul(out=po[:, :NW], lhsT=w3s_0[:],
                                 rhs=h2a[:, :NW], start=True, stop=False)
                nc.tensor.matmul(out=po[:, :NW], lhsT=w3s_1[:],
                                 rhs=h2b[:, :NW], start=False, stop=True)
                _evac(nc, "og", oall[:, n0 : n0 + NW], po[:, :NW],
                      b3s[:, 0:1], relu=False)

            store_dma(
                out=outT_d[:, 0:flush_hi], in_=oall[:, 0:flush_hi]
            )

    nc.finalize()
    return nc


def _pack_windows(profile):
    """DP over node ranks: choose window boundaries (<=W nodes each) to
    minimize chunk+window cost. profile = descending per-rank edge-count
    upper bound. Returns list of per-window node spans (r0, r1)."""
    n = len(profile)
    csum = np.zeros(n + 1, np.float64)
    csum[1:] = np.cumsum(profile)
    INF = float("inf")
    dp = np.full(n + 1, INF)
    prev = np.zeros(n + 1, np.int32)
    dp[0] = 0.0
    wplan = W - PLAN_SLACK  # reserve slots per window for tail fillers
    for r1 in range(1, n + 1):
        best = INF
        barg = r1 - 1
        for r0 in range(max(0, r1 - wplan), r1):
            if dp[r0] == INF:
                continue
            e = csum[r1] - csum[r0]
            # windows share tail chunks in 64-edge halves, so capacity is
            # 64-granular at half the per-chunk cost
            cost = dp[r0] + CHUNK_COST * 0.5 * max((e + 63) // 64, 1) + WINDOW_COST
            if cost < best:
                best = cost
                barg = r0
        dp[r1] = best
        prev[r1] = barg
    spans = []
    r = n
    while r > 0:
        spans.append((int(prev[r]), r))
        r = int(prev[r])
    spans.reverse()
    return spans


def _fill(cnts, cap, n_windows):
    """Head+tail fill: place descending-count nodes (head pointer) into
    windows (<=W slots, cap[j] edge budget); when the next head node
    overflows the remaining budget, plug the gap with the globally
    smallest remaining nodes (tail pointer), which fit nearly exactly.
    Returns placement arrays + per-window usage."""
    n = len(cnts)
    plorder = np.empty(n, np.int64)
    win_of_place = np.empty(n, np.int64)
    idx_of_place = np.empty(n, np.int64)
    used = np.zeros(n_windows, np.int64)
    nplace = np.zeros(n_windows, np.int64)
    hp = 0
    tp = n - 1
    p = 0
    for j in range(n_windows):
        if hp > tp:
            break
        e_used = 0
        slots = 0

        def place(rk, e):
            nonlocal p, slots
            plorder[p] = rk
            win_of_place[p] = j
            idx_of_place[p] = slots
            slots += 1
            p += 1
            return e + cnts[rk]

        while slots < W and hp <= tp and e_used + cnts[hp] <= cap[j]:
            e_used = place(hp, e_used)
            hp += 1
        # gap fill with the smallest remaining nodes
        while slots < W and hp <= tp and e_used + cnts[tp] <= cap[j]:
            e_used = place(tp, e_used)
            tp -= 1
        used[j] = e_used
        nplace[j] = slots
    assert p == n, "core overflowed the window plan"
    return plorder, win_of_place, idx_of_place, used, nplace


def _host_prep(x, edge_index, edge_attr):
    """Sort/scale/pad edges; returns (META, per-core input arrays,
    per-core slot->global-node maps)."""
    col = np.asarray(edge_index)[1].astype(np.int64)
    x = np.asarray(x, dtype=np.float32)
    counts = np.bincount(col, minlength=N_NODES).astype(np.int64)
    scale = (1.0 / np.maximum(counts, 1)).astype(np.float32)

    eorder = np.argsort(col, kind="stable")
    col_s = col[eorder]
    attr_s = np.asarray(edge_attr, dtype=np.float32)[eorder]
    attr_s = attr_s * scale[col_s][:, None]
    attr_s8 = attr_s.astype(ml_dtypes.float8_e4m3)
    # edge start offset of each node in the dst-sorted arrays
    estart = np.zeros(N_NODES + 1, np.int64)
    estart[1:] = np.cumsum(counts)

    # per-core descending-count node order + rank-wise max profile
    nodeperm = np.empty((N_CORES, NPC_REAL), np.int64)   # rank -> local node
    cnt_sorted = np.empty((N_CORES, NPC_REAL), np.int64)
    for c in range(N_CORES):
        cnt_c = counts[c * NPC_REAL : (c + 1) * NPC_REAL]
        p = np.argsort(-cnt_c, kind="stable")
        nodeperm[c] = p
        cnt_sorted[c] = cnt_c[p]
    # plan on the rank-wise mean profile; per-core greedy fill spills
    # overflow forward, spare windows at the end absorb the tail
    profile = cnt_sorted.mean(axis=0)

    spans = _pack_windows(profile)
    n_spare = 32
    spans = spans + [(NPC_REAL, NPC_REAL)] * n_spare
    n_windows = len(spans)

    # planned per-window capacity (64-edge granular)
    cap0 = np.zeros(n_windows, np.int64)
    for j, (r0, r1) in enumerate(spans):
        s = float(profile[r0:r1].sum())
        cap0[j] = max(int(np.ceil(s / 64)) * 64, 64)

    # fill every core against the planned caps, then shrink each cap to
    # the cross-core max usage (the fill provably replays identically
    # under shrunk caps, so the placements stay valid)
    fills = [_fill(cnt_sorted[c], cap0, n_windows) for c in range(N_CORES)]
    used_max = np.max([f[3] for f in fills], axis=0)
    placed_any = np.max([f[4] for f in fills], axis=0) > 0
    cap = ((used_max + 63) // 64) * 64
    cap[placed_any & (cap == 0)] = 64

    # per-window chunks: full chunks + optional shared half chunk (two
    # half-windows in a group share one 128-row chunk, first in
    # partitions 0:64, second in 64:128)
    fullc = cap // 128
    is_half = (cap % 128) > 0

    gsizes = _group_plan(n_windows)
    gbounds = []
    w = 0
    for gs in gsizes:
        gbounds.append((w, w + gs))
        w += gs

    # chunk allocation + half pairing (within groups, so group DMA slices
    # stay contiguous)
    wparts = [[] for _ in range(n_windows)]
    gchunks = []
    colc = 0
    for w0, w1 in gbounds:
        co0 = colc
        pend = None
        for j in range(w0, w1):
            for _ in range(fullc[j]):
                wparts[j].append((colc, 0, 128))
                colc += 1
            if is_half[j]:
                if pend is None:
                    pend = colc
                    wparts[j].append((colc, 0, 64))
                    colc += 1
                else:
                    wparts[j].append((pend, 64, 128))
                    pend = None
        gchunks.append((co0, colc))
    NCH = colc
    E_pad = NCH * P
    NPC = n_windows * W

    capstart = np.zeros(n_windows + 1, np.int64)
    capstart[1:] = np.cumsum(cap)
    pos_all = np.empty(int(cap.sum()), np.int64)
    for j in range(n_windows):
        o = capstart[j]
        for c, p0, p1 in wparts[j]:
            pos_all[o : o + p1 - p0] = np.arange(c * P + p0, c * P + p1)
            o += p1 - p0

    META = (
        tuple(gbounds),
        tuple(tuple(p) for p in wparts),
        tuple(gchunks),
        NCH,
    )

    per_core = []
    slot_node = []  # per core: slot -> global node id (-1 empty)
    for c in range(N_CORES):
        cnts = cnt_sorted[c]
        plorder, win_of_place, idx_of_place, _, _ = fills[c]
        cnts_p = cnts[plorder]
        newwin = np.ones(NPC_REAL, bool)
        newwin[1:] = win_of_place[1:] != win_of_place[:-1]
        pre = np.cumsum(cnts_p) - cnts_p
        wstart = np.maximum.accumulate(np.where(newwin, pre, -1))
        prefix_in_win = pre - wstart
        rank_base = capstart[win_of_place] + prefix_in_win

        # edges in placement order
        lnode = nodeperm[c][plorder]           # placement -> local node
        gnode = lnode + c * NPC_REAL
        src0 = estart[gnode]                   # first edge (dst-sorted)
        total = int(cnts_p.sum())
        src_idx = np.concatenate(
            [np.arange(src0[i], src0[i] + cnts_p[i]) for i in range(NPC_REAL)]
        ) if total else np.empty(0, np.int64)
        within = np.arange(total) - np.repeat(pre, cnts_p)
        edest = pos_all[np.repeat(rank_base, cnts_p) + within]

        attr_pad = np.zeros((E_pad, D), ml_dtypes.float8_e4m3)
        attr_pad[edest] = attr_s8[src_idx]
        attr8 = np.ascontiguousarray(
            attr_pad.reshape(NCH, P, D).transpose(1, 0, 2).reshape(P, NCH * D)
        )

        dstrel = np.full((E_pad,), 200.0, np.float16)
        dstrel[edest] = np.repeat(idx_of_place, cnts_p).astype(np.float16)
        dstrelT = np.ascontiguousarray(dstrel.reshape(NCH, P).T)

        # node features + slot map
        slot = win_of_place * W + idx_of_place
        smap = np.full(NPC, -1, np.int64)
        smap[slot] = gnode
        xT = np.zeros((NPC, D), np.float16)
        xT[slot] = x[gnode].astype(np.float16)
        xT = np.ascontiguousarray(xT.T)

        per_core.append({"attr8": attr8, "dstrelT": dstrelT, "x16": xT})
        slot_node.append(smap)
    return META, per_core, slot_node


def _build_consts(b1, b2, b3):
    consts = np.zeros((P, 5), np.float32)
    consts[:, 0] = b1[:P]
    consts[:, 1] = b1[P:]
    consts[:, 2] = b2[:P]
    consts[:, 3] = b2[P:]
    consts[:, 4] = b3
    return consts


def _build_wts(W1, W2, W3):
    wts = np.empty((P, 4 * HIDDEN + 2 * DOUT), np.float16)
    wts[:, 0:HIDDEN] = W1[:P]
    wts[:, HIDDEN : 2 * HIDDEN] = W1[P:]
    wts[:, 2 * HIDDEN : 3 * HIDDEN] = W2[:P]
    wts[:, 3 * HIDDEN : 4 * HIDDEN] = W2[P:]
    wts[:, 4 * HIDDEN : 4 * HIDDEN + DOUT] = W3[:P]
    wts[:, 4 * HIDDEN + DOUT : 4 * HIDDEN + 2 * DOUT] = W3[P:]
    return wts


def _build_c16(META, dstrelT):
    """fp16 consts row-block: iota ramp | dstrel."""
    gbounds, wparts, gchunks, NCH = META
    _, oh_batches = _oh_plan(gbounds, gchunks)
    CB_OH = max(top - base for base, top, _ in oh_batches)
    c16 = np.empty((P, CB_OH * W + NCH), np.float16)
    c16[:, 0 : CB_OH * W] = np.tile(
        np.arange(W, dtype=np.float16), CB_OH
    )[None, :]
    c16[:, CB_OH * W :] = dstrelT
    return c16


def _make_in_maps(META, per_core, b1, b2, b3, W1, W2, W3):
    consts = _build_consts(b1, b2, b3)
    wts = _build_wts(W1, W2, W3)
    return [
        {
            "attr8": pc["attr8"].view(np.uint8),
            "x16": pc["x16"],
            "c16": _build_c16(META, pc["dstrelT"]),
            "consts": consts,
            "wts": wts,
        }
        for pc in per_core
    ]


def kernel(x, edge_index, edge_attr, W1, b1, W2, b2, W3, b3):
    META, per_core, slot_node = _host_prep(x, edge_index, edge_attr)

    if META not in _prog_cache:
        _prog_cache[META] = _build_program(META)
    nc = _prog_cache[META]

    in_maps = _make_in_maps(
        META, per_core,
        np.asarray(b1, np.float32), np.asarray(b2, np.float32),
        np.asarray(b3, np.float32),
        np.asarray(W1, np.float32), np.asarray(W2, np.float32),
        np.asarray(W3, np.float32),
    )
    res = run_bass_kernel_spmd(nc, in_maps, core_ids=list(range(N_CORES)))

    out = np.empty((N_NODES, DOUT), np.float32)
    for c in range(N_CORES):
        o = res.results[c]["outT"].T.astype(np.float32)
        smap = slot_node[c]
        m = smap >= 0
        out[smap[m]] = o[m]
    return out


# revision 91
# speedup vs baseline: 1.6859x; 1.0031x over previous
"""NodeNet GNN message-passing kernel for 8 Trainium2 NeuronCores.

Strategy (per sharding hint): shard nodes across the 8 cores; partition
edges by destination node on the host so the scatter-mean is device-local.

v3 — fp8 edge stream + padding-minimizing packing + pipelined groups:
  - Host sorts each core's 12,500 nodes by descending edge count and sorts
    edges by destination; edge rows are pre-scaled by 1/count(dst) so the
    device segment-sum directly yields the mean, then cast to fp8 e4m3
    (end-to-end absmax error ~9e-3 vs the 2e-2 gate; the scatter-mean
    averages the quantization noise before the MLP sees it).
  - Windows hold up to W=16 node slots; a host DP over the rank-wise mean
    count profile picks window spans whose edge lists land near 64-edge
    half-chunk boundaries. Two half-windows in a group share one 128-row
    chunk (partitions 0:64 / 64:128). Each core packs nodes head-first and
    plugs boundary gaps with its smallest remaining nodes (tail fill);
    caps then shrink to the cross-core max usage (the fill replays
    identically), leaving ~5% edge padding. One shared SPMD chunk plan.
  - Per group (~32 windows, 512 node slots): ScalarE pre-expands dst-rel
    to a packed wide tile (a stride-0 broadcast operand would deny DVE its
    2x mode), VectorE builds the one-hot with ONE 2x is_equal against an
    iota ramp, and TensorE contracts chunk-by-chunk (fp8 lhsT x fp16 rhs,
    fp32 accumulate) into per-window slices of one PSUM bank tile.
  - The 3-layer MLP runs feature-major per group; PSUM evacuations are
    split ScalarE (mean, h1a, h2a, out) / VectorE (h1b, h2b) — chosen by
    sweep: in-order queue coupling, not busy balance, sets the cadence.
  - Groups are processed smallest-first (reversed plan order); outputs
    accumulate in one SBUF tile and are flushed as large slabs from the
    idle Pool engine's SWDGE queue, 2 groups behind, so stores can never
    head-of-line block the SP edge-stream queue.

Cost-model timeline (per core): ~97 us DMA (34 MB/core at 360 GB/s, 89%
occupancy), ACT ~92 us, PE ~68 us, DVE ~49 us; wall 108.4 us vs the
182.2 us session-start baseline.
"""

import numpy as np
import ml_dtypes

import concourse.bacc as bacc
import concourse.mybir as mybir
import concourse.tile as tile
from concourse.bass_utils import run_bass_kernel_spmd

P = 128                    # partitions / matmul contraction tile
D = 128                    # node & edge feature dim
HIDDEN = 256
DOUT = 128
N_NODES = 100000
N_CORES = 8
NPC_REAL = 12500           # real nodes per core
W = 16                     # node slots per window (one-hot width)

# marginal cost weights for the host packing DP (ns, from the TRN2 cost
# model): one 128-edge chunk costs DMA 45.5 + PE 6.7 + DVE 16.7; one
# window costs 16 node slots of MLP/DMA work
CHUNK_COST = 70.0
WINDOW_COST = 90.0
PLAN_SLACK = 0     # slots per window the DP leaves for tail fillers

_prog_cache: dict = {}

# engine assignment for the six PSUM evacuations (chain cadence tuning)
EVAC = {"mean": "A", "h1a": "A", "h1b": "V", "h2a": "A", "h2b": "V", "og": "A"}
EXP_ENG = "A"     # dstrel expand: "A" ScalarE, "P" Pool, "N" none (1x)
STORE_ENG = "P"   # slab stores: "P" Pool SWDGE, "S" SP queue
STORE_LAG = 1     # groups of lag before a finished slab is stored
STORE_SLAB = 1    # flush cadence in processed groups
OH_PAIR = 1       # processed groups sharing one expand+is_equal pair
ATTR_BUFS = 8
OH_BUFS = 5


def _oh_plan(gbounds, gchunks):
    """Processing order (reversed plan) and one-hot batches: OH_PAIR
    consecutive processed groups share one contiguous chunk span."""
    porder = list(range(len(gbounds)))[::-1]
    batches = []
    for i in range(0, len(porder), OH_PAIR):
        mem = porder[i : i + OH_PAIR]
        base = min(gchunks[g][0] for g in mem)
        top = max(gchunks[g][1] for g in mem)
        batches.append((base, top, tuple(mem)))
    return porder, batches

f32 = mybir.dt.float32
f16 = mybir.dt.float16
f8 = mybir.dt.float8e4

Relu = mybir.ActivationFunctionType.Relu
Ident = mybir.ActivationFunctionType.Identity


def _evac(nc, key, out, in_, bias, relu=True):
    """PSUM->SBUF evacuation with bias (+relu) on ACT or DVE per EVAC.
    Mode "S" splits the columns: ACT takes the first half, DVE the
    second, halving that chain stage's latency."""
    if EVAC[key] == "S":
        ncols = out.shape[-1]
        h = (ncols // 2 + 1) & ~1
        saved = dict(EVAC)
        try:
            EVAC[key] = "A"
            _evac(nc, key, out[:, :h], in_[:, :h], bias, relu)
            EVAC[key] = "V"
            _evac(nc, key, out[:, h:], in_[:, h:], bias, relu)
        finally:
            EVAC.update(saved)
        return
    if EVAC[key] == "A":
        if bias is None:
            nc.scalar.copy(out=out, in_=in_)
        else:
            nc.scalar.activation(out=out, in_=in_,
                                 func=(Relu if relu else Ident), bias=bias)
    else:
        if bias is None:
            nc.vector.tensor_copy(out=out, in_=in_)
        elif relu:
            nc.vector.tensor_scalar(
                out=out, in0=in_, scalar1=bias, scalar2=0.0,
                op0=mybir.AluOpType.add, op1=mybir.AluOpType.max,
            )
        else:
            nc.vector.tensor_scalar(
                out=out, in0=in_, scalar1=bias, scalar2=None,
                op0=mybir.AluOpType.add,
            )


def _group_plan(n_windows):
    """Group sizes in windows: small groups first (compute starts early),
    steady-state 32-window groups (512 node slots), aggressively tapered
    tail (the backlog drains at per-group chain latency, so the last
    chains must be short)."""
    # groups are PROCESSED in reversed plan order; both ends taper (fast
    # pipeline fill at the processing start, short chains at the drain)
    gpw = 512 // W  # windows per full 512-column group
    tail = [(3 * gpw) // 4, gpw // 2, gpw // 4]
    gsizes = list(tail[::-1])
    rem = n_windows - sum(gsizes) - sum(tail)
    while rem >= gpw:
        gsizes.append(gpw)
        rem -= gpw
    if rem:
        gsizes.append(rem)
    return gsizes + tail


def _build_program(META):
    """META = (gbounds, wparts, gchunks, NCH): per-group window bounds,
    per-window matmul parts (chunk col, partition lo, hi), per-group
    chunk ranges; identical across cores (SPMD)."""
    gbounds, wparts, gchunks, NCH = META
    n_windows = len(wparts)
    NPC = n_windows * W
    CBG_max = max(c1 - c0 for c0, c1 in gchunks)
    NW_max = max(w1 - w0 for w0, w1 in gbounds) * W
    porder, oh_batches = _oh_plan(gbounds, gchunks)
    CB_OH = max(top - base for base, top, _ in oh_batches)
    batch_of = {}
    for bi, (_, _, mem) in enumerate(oh_batches):
        for g in mem:
            batch_of[g] = bi

    nc = bacc.Bacc(None)
    attr8_d = nc.dram_tensor("attr8", [P, NCH * D], f8, kind="ExternalInput")
    x16_d = nc.dram_tensor("x16", [P, NPC], f16, kind="ExternalInput")
    # fp16 consts: iota ramp (CB_OH*W) | dstrel (NCH)
    c16_d = nc.dram_tensor("c16", [P, CB_OH * W + NCH], f16, kind="ExternalInput")
    consts_d = nc.dram_tensor("consts", [P, 5], f32, kind="ExternalInput")
    wts_d = nc.dram_tensor("wts", [P, 4 * HIDDEN + 2 * DOUT], f16,
                           kind="ExternalInput")
    outT_d = nc.dram_tensor("outT", [P, NPC], f16, kind="ExternalOutput")

    with tile.TileContext(nc) as tc:
        with (
            tc.tile_pool(name="const", bufs=1) as cpool,
            tc.tile_pool(name="attr", bufs=ATTR_BUFS) as apool,
            tc.tile_pool(name="x", bufs=4) as xpool,
            tc.tile_pool(name="oh", bufs=OH_BUFS) as ohpool,
            tc.tile_pool(name="dw", bufs=3) as dwpool,
            tc.tile_pool(name="acts", bufs=3) as actpool,
            tc.tile_pool(name="pbin", bufs=2, space="PSUM") as pbin,
            tc.tile_pool(name="pmlp", bufs=1, space="PSUM") as pmlp,
            tc.tile_pool(name="ppo", bufs=2, space="PSUM") as ppo,
        ):
            cs = cpool.tile([P, 5], f32, tag="consts")
            ws = cpool.tile([P, 4 * HIDDEN + 2 * DOUT], f16, tag="wts")
            c16 = cpool.tile([P, CB_OH * W + NCH], f16, tag="c16")
            w1s_0 = ws[:, 0:HIDDEN]
            w1s_1 = ws[:, HIDDEN : 2 * HIDDEN]
            w2s_0 = ws[:, 2 * HIDDEN : 3 * HIDDEN]
            w2s_1 = ws[:, 3 * HIDDEN : 4 * HIDDEN]
            w3s_0 = ws[:, 4 * HIDDEN : 4 * HIDDEN + DOUT]
            w3s_1 = ws[:, 4 * HIDDEN + DOUT : 4 * HIDDEN + 2 * DOUT]
            b1s_0 = cs[:, 0:1]
            b1s_1 = cs[:, 1:2]
            b2s_0 = cs[:, 2:3]
            b2s_1 = cs[:, 3:4]
            b3s = cs[:, 4:5]
            it16 = c16[:, 0 : CB_OH * W]
            dstrel_s = c16[:, CB_OH * W : CB_OH * W + NCH]
            oall = cpool.tile([P, NPC], f16, tag="oall")

            # deferred slab stores: flush finished output columns with a
            # lag so the store never races its own evacuations
            SLAB = STORE_SLAB
            LAG = STORE_LAG
            store_dma = (nc.gpsimd.dma_start if STORE_ENG == "P"
                         else nc.sync.dma_start)

            def build_oh(bi):
                """One-hot for batch bi (OH_PAIR processed groups' chunk
                span) — depends only on c16. ScalarE expands dstrel to a
                packed wide tile first: a broadcast (stride-0) operand
                would deny the DVE is_equal its 2x mode."""
                bco0, bco1, _ = oh_batches[bi]
                bCB = bco1 - bco0
                oh = ohpool.tile([P, CB_OH * W], f16, tag="oh")
                if EXP_ENG == "N":
                    # plain 1x is_equal with the broadcast operand (all DVE)
                    nc.vector.tensor_tensor(
                        out=oh[:, : bCB * W].rearrange("p (c m) -> p c m", m=W),
                        in0=dstrel_s[:, bco0:bco1].to_broadcast([P, bCB, W]),
                        in1=it16[:, : bCB * W].rearrange("p (c m) -> p c m", m=W),
                        op=mybir.AluOpType.is_equal,
                    )
                    return oh
                dw = dwpool.tile([P, CB_OH * W], f16, tag="dw")
                exp_copy = {"A": nc.scalar.copy, "P": nc.gpsimd.tensor_copy,
                            "V": nc.vector.tensor_copy}[EXP_ENG]
                exp_copy(
                    out=dw[:, : bCB * W].rearrange("p (c m) -> p c m", m=W),
                    in_=dstrel_s[:, bco0:bco1].to_broadcast([P, bCB, W]),
                )
                nc.vector.tensor_tensor(
                    out=oh[:, : bCB * W],
                    in0=dw[:, : bCB * W],
                    in1=it16[:, : bCB * W],
                    op=mybir.AluOpType.is_equal,
                )
                return oh

            # process groups smallest-first (reversed plan order): the big
            # groups' delivery cadence exceeds their compute cadence, so the
            # compute backlog shrinks toward the end and the drain collapses
            # to roughly one group's chain latency
            oh_cache = {}   # batch index -> oh tile
            flush_hi = NPC
            first = True
            for gi, g in enumerate(porder):
                w0, w1 = gbounds[g]
                NW = (w1 - w0) * W
                n0 = w0 * W
                co0, co1 = gchunks[g]
                CBg = co1 - co0
                if CBg == 0:
                    # no core placed any node here (shrunk spare windows)
                    continue

                at8 = apool.tile([P, CBG_max * D], f8, tag="attr")
                nc.sync.dma_start(
                    out=at8[:, : CBg * D], in_=attr8_d[:, co0 * D : co1 * D]
                )
                if first:
                    first = False
                    nc.sync.dma_start(out=c16[:], in_=c16_d[:, :])
                    nc.sync.dma_start(out=cs[:], in_=consts_d[:, :])
                    nc.sync.dma_start(out=ws[:], in_=wts_d[:, :])
                xg = xpool.tile([P, NW_max], f16, tag="x")
                nc.sync.dma_start(out=xg[:, :NW], in_=x16_d[:, n0 : n0 + NW])
                if gi >= SLAB + LAG and gi % SLAB == 0:
                    f0 = gbounds[porder[gi - LAG]][0] * W
                    store_dma(
                        out=outT_d[:, f0:flush_hi], in_=oall[:, f0:flush_hi]
                    )
                    flush_hi = f0

                bi = batch_of[g]
                if bi not in oh_cache:
                    oh_cache = {bi: build_oh(bi)}
                oh = oh_cache[bi]
                ob = oh_batches[bi][0]  # batch chunk base

                pm = pbin.tile([P, NW_max], f32, tag="pm")
                for w in range(w0, w1):
                    parts = wparts[w]
                    sw = w - w0
                    for i, (c, p0, p1) in enumerate(parts):
                        lc = c - co0
                        lo = c - ob
                        nc.tensor.matmul(
                            out=pm[:, sw * W : (sw + 1) * W],
                            lhsT=at8[p0:p1, lc * D : (lc + 1) * D],
                            rhs=oh[p0:p1, lo * W : (lo + 1) * W],
                            start=(i == 0),
                            stop=(i == len(parts) - 1),
                        )
                mean_g = actpool.tile([P, NW_max], f16, tag="mean_g")
                _evac(nc, "mean", mean_g[:, :NW], pm[:, :NW], None)

                # --- MLP over this group (feature-major) ---
                ph1a = pmlp.tile([P, NW_max], f32, tag="h1a")
                ph1b = pmlp.tile([P, NW_max], f32, tag="h1b")
                nc.tensor.matmul(out=ph1a[:, :NW], lhsT=w1s_0[:, 0:P],
                                 rhs=xg[:, :NW], start=True, stop=False)
                nc.tensor.matmul(out=ph1a[:, :NW], lhsT=w1s_1[:, 0:P],
                                 rhs=mean_g[:, :NW], start=False, stop=True)
                nc.tensor.matmul(out=ph1b[:, :NW], lhsT=w1s_0[:, P:HIDDEN],
                                 rhs=xg[:, :NW], start=True, stop=False)
                nc.tensor.matmul(out=ph1b[:, :NW], lhsT=w1s_1[:, P:HIDDEN],
                                 rhs=mean_g[:, :NW], start=False, stop=True)
                h1a = actpool.tile([P, NW_max], f16, tag="h1a_s")
                h1b = actpool.tile([P, NW_max], f16, tag="h1b_s")
                # paired evacuations on different engines run in parallel
                # instead of back-to-back on one in-order queue
                _evac(nc, "h1a", h1a[:, :NW], ph1a[:, :NW], b1s_0[:, 0:1])
                _evac(nc, "h1b", h1b[:, :NW], ph1b[:, :NW], b1s_1[:, 0:1])

                ph2a = pmlp.tile([P, NW_max], f32, tag="h2a")
                ph2b = pmlp.tile([P, NW_max], f32, tag="h2b")
                nc.tensor.matmul(out=ph2a[:, :NW], lhsT=w2s_0[:, 0:P],
                                 rhs=h1a[:, :NW], start=True, stop=False)
                nc.tensor.matmul(out=ph2a[:, :NW], lhsT=w2s_1[:, 0:P],
                                 rhs=h1b[:, :NW], start=False, stop=True)
                nc.tensor.matmul(out=ph2b[:, :NW], lhsT=w2s_0[:, P:HIDDEN],
                                 rhs=h1a[:, :NW], start=True, stop=False)
                nc.tensor.matmul(out=ph2b[:, :NW], lhsT=w2s_1[:, P:HIDDEN],
                                 rhs=h1b[:, :NW], start=False, stop=True)
                h2a = actpool.tile([P, NW_max], f16, tag="h2a_s")
                h2b = actpool.tile([P, NW_max], f16, tag="h2b_s")
                _evac(nc, "h2a", h2a[:, :NW], ph2a[:, :NW], b2s_0[:, 0:1])
                _evac(nc, "h2b", h2b[:, :NW], ph2b[:, :NW], b2s_1[:, 0:1])

                po = ppo.tile([P, NW_max], f32, tag="po")
                nc.tensor.matmul(out=po[:, :NW], lhsT=w3s_0[:],
                                 rhs=h2a[:, :NW], start=True, stop=False)
                nc.tensor.matmul(out=po[:, :NW], lhsT=w3s_1[:],
                                 rhs=h2b[:, :NW], start=False, stop=True)
                _evac(nc, "og", oall[:, n0 : n0 + NW], po[:, :NW],
                      b3s[:, 0:1], relu=False)

            store_dma(
                out=outT_d[:, 0:flush_hi], in_=oall[:, 0:flush_hi]
            )

    nc.finalize()
    return nc


def _pack_windows(profile):
    """DP over node ranks: choose window boundaries (<=W nodes each) to
    minimize chunk+window cost. profile = descending per-rank edge-count
    upper bound. Returns list of per-window node spans (r0, r1)."""
    n = len(profile)
    csum = np.zeros(n + 1, np.float64)
    csum[1:] = np.cumsum(profile)
    INF = float("inf")
    dp = np.full(n + 1, INF)
    prev = np.zeros(n + 1, np.int32)
    dp[0] = 0.0
    wplan = W - PLAN_SLACK  # reserve slots per window for tail fillers
    for r1 in range(1, n + 1):
        best = INF
        barg = r1 - 1
        for r0 in range(max(0, r1 - wplan), r1):
            if dp[r0] == INF:
                continue
            e = csum[r1] - csum[r0]
            # windows share tail chunks in 64-edge halves, so capacity is
            # 64-granular at half the per-chunk cost
            cost = dp[r0] + CHUNK_COST * 0.5 * max((e + 63) // 64, 1) + WINDOW_COST
            if cost < best:
                best = cost
                barg = r0
        dp[r1] = best
        prev[r1] = barg
    spans = []
    r = n
    while r > 0:
        spans.append((int(prev[r]), r))
        r = int(prev[r])
    spans.reverse()
    return spans


def _fill(cnts, cap, n_windows):
    """Head+tail fill: place descending-count nodes (head pointer) into
    windows (<=W slots, cap[j] edge budget); when the next head node
    overflows the remaining budget, plug the gap with the globally
    smallest remaining nodes (tail pointer), which fit nearly exactly.
    Returns placement arrays + per-window usage."""
    n = len(cnts)
    plorder = np.empty(n, np.int64)
    win_of_place = np.empty(n, np.int64)
    idx_of_place = np.empty(n, np.int64)
    used = np.zeros(n_windows, np.int64)
    nplace = np.zeros(n_windows, np.int64)
    hp = 0
    tp = n - 1
    p = 0
    for j in range(n_windows):
        if hp > tp:
            break
        e_used = 0
        slots = 0

        def place(rk, e):
            nonlocal p, slots
            plorder[p] = rk
            win_of_place[p] = j
            idx_of_place[p] = slots
            slots += 1
            p += 1
            return e + cnts[rk]

        while slots < W and hp <= tp and e_used + cnts[hp] <= cap[j]:
            e_used = place(hp, e_used)
            hp += 1
        # gap fill with the smallest remaining nodes
        while slots < W and hp <= tp and e_used + cnts[tp] <= cap[j]:
            e_used = place(tp, e_used)
            tp -= 1
        used[j] = e_used
        nplace[j] = slots
    assert p == n, "core overflowed the window plan"
    return plorder, win_of_place, idx_of_place, used, nplace


def _host_prep(x, edge_index, edge_attr):
    """Sort/scale/pad edges; returns (META, per-core input arrays,
    per-core slot->global-node maps)."""
    col = np.asarray(edge_index)[1].astype(np.int64)
    x = np.asarray(x, dtype=np.float32)
    counts = np.bincount(col, minlength=N_NODES).astype(np.int64)
    scale = (1.0 / np.maximum(counts, 1)).astype(np.float32)

    eorder = np.argsort(col, kind="stable")
    col_s = col[eorder]
    attr_s = np.asarray(edge_attr, dtype=np.float32)[eorder]
    attr_s = attr_s * scale[col_s][:, None]
    attr_s8 = attr_s.astype(ml_dtypes.float8_e4m3)
    # edge start offset of each node in the dst-sorted arrays
    estart = np.zeros(N_NODES + 1, np.int64)
    estart[1:] = np.cumsum(counts)

    # per-core descending-count node order + rank-wise max profile
    nodeperm = np.empty((N_CORES, NPC_REAL), np.int64)   # rank -> local node
    cnt_sorted = np.empty((N_CORES, NPC_REAL), np.int64)
    for c in range(N_CORES):
        cnt_c = counts[c * NPC_REAL : (c + 1) * NPC_REAL]
        p = np.argsort(-cnt_c, kind="stable")
        nodeperm[c] = p
        cnt_sorted[c] = cnt_c[p]
    # plan on the rank-wise mean profile; per-core greedy fill spills
    # overflow forward, spare windows at the end absorb the tail
    profile = cnt_sorted.mean(axis=0)

    spans = _pack_windows(profile)
    n_spare = 32
    spans = spans + [(NPC_REAL, NPC_REAL)] * n_spare
    n_windows = len(spans)

    # planned per-window capacity (64-edge granular)
    cap0 = np.zeros(n_windows, np.int64)
    for j, (r0, r1) in enumerate(spans):
        s = float(profile[r0:r1].sum())
        cap0[j] = max(int(np.ceil(s / 64)) * 64, 64)

    # fill every core against the planned caps, then shrink each cap to
    # the cross-core max usage (the fill provably replays identically
    # under shrunk caps, so the placements stay valid)
    fills = [_fill(cnt_sorted[c], cap0, n_windows) for c in range(N_CORES)]
    used_max = np.max([f[3] for f in fills], axis=0)
    placed_any = np.max([f[4] for f in fills], axis=0) > 0
    cap = ((used_max + 63) // 64) * 64
    cap[placed_any & (cap == 0)] = 64

    # per-window chunks: full chunks + optional shared half chunk (two
    # half-windows in a group share one 128-row chunk, first in
    # partitions 0:64, second in 64:128)
    fullc = cap // 128
    is_half = (cap % 128) > 0

    gsizes = _group_plan(n_windows)
    gbounds = []
    w = 0
    for gs in gsizes:
        gbounds.append((w, w + gs))
        w += gs

    # chunk allocation + half pairing (within groups, so group DMA slices
    # stay contiguous)
    wparts = [[] for _ in range(n_windows)]
    gchunks = []
    colc = 0
    for w0, w1 in gbounds:
        co0 = colc
        pend = None
        for j in range(w0, w1):
            for _ in range(fullc[j]):
                wparts[j].append((colc, 0, 128))
                colc += 1
            if is_half[j]:
                if pend is None:
                    pend = colc
                    wparts[j].append((colc, 0, 64))
                    colc += 1
                else:
                    wparts[j].append((pend, 64, 128))
                    pend = None
        gchunks.append((co0, colc))
    NCH = colc
    E_pad = NCH * P
    NPC = n_windows * W

    capstart = np.zeros(n_windows + 1, np.int64)
    capstart[1:] = np.cumsum(cap)
    pos_all = np.empty(int(cap.sum()), np.int64)
    for j in range(n_windows):
        o = capstart[j]
        for c, p0, p1 in wparts[j]:
            pos_all[o : o + p1 - p0] = np.arange(c * P + p0, c * P + p1)
            o += p1 - p0

    META = (
        tuple(gbounds),
        tuple(tuple(p) for p in wparts),
        tuple(gchunks),
        NCH,
    )

    per_core = []
    slot_node = []  # per core: slot -> global node id (-1 empty)
    for c in range(N_CORES):
        cnts = cnt_sorted[c]
        plorder, win_of_place, idx_of_place, _, _ = fills[c]
        cnts_p = cnts[plorder]
        newwin = np.ones(NPC_REAL, bool)
        newwin[1:] = win_of_place[1:] != win_of_place[:-1]
        pre = np.cumsum(cnts_p) - cnts_p
        wstart = np.maximum.accumulate(np.where(newwin, pre, -1))
        prefix_in_win = pre - wstart
        rank_base = capstart[win_of_place] + prefix_in_win

        # edges in placement order
        lnode = nodeperm[c][plorder]           # placement -> local node
        gnode = lnode + c * NPC_REAL
        src0 = estart[gnode]                   # first edge (dst-sorted)
        total = int(cnts_p.sum())
        src_idx = np.concatenate(
            [np.arange(src0[i], src0[i] + cnts_p[i]) for i in range(NPC_REAL)]
        ) if total else np.empty(0, np.int64)
        within = np.arange(total) - np.repeat(pre, cnts_p)
        edest = pos_all[np.repeat(rank_base, cnts_p) + within]

        attr_pad = np.zeros((E_pad, D), ml_dtypes.float8_e4m3)
        attr_pad[edest] = attr_s8[src_idx]
        attr8 = np.ascontiguousarray(
            attr_pad.reshape(NCH, P, D).transpose(1, 0, 2).reshape(P, NCH * D)
        )

        dstrel = np.full((E_pad,), 200.0, np.float16)
        dstrel[edest] = np.repeat(idx_of_place, cnts_p).astype(np.float16)
        dstrelT = np.ascontiguousarray(dstrel.reshape(NCH, P).T)

        # node features + slot map
        slot = win_of_place * W + idx_of_place
        smap = np.full(NPC, -1, np.int64)
        smap[slot] = gnode
        xT = np.zeros((NPC, D), np.float16)
        xT[slot] = x[gnode].astype(np.float16)
        xT = np.ascontiguousarray(xT.T)

        per_core.append({"attr8": attr8, "dstrelT": dstrelT, "x16": xT})
        slot_node.append(smap)
    return META, per_core, slot_node


def _build_consts(b1, b2, b3):
    consts = np.zeros((P, 5), np.float32)
    consts[:, 0] = b1[:P]
    consts[:, 1] = b1[P:]
    consts[:, 2] = b2[:P]
    consts[:, 3] = b2[P:]
    consts[:, 4] = b3
    return consts


def _build_wts(W1, W2, W3):
    wts = np.empty((P, 4 * HIDDEN + 2 * DOUT), np.float16)
    wts[:, 0:HIDDEN] = W1[:P]
    wts[:, HIDDEN : 2 * HIDDEN] = W1[P:]
    wts[:, 2 * HIDDEN : 3 * HIDDEN] = W2[:P]
    wts[:, 3 * HIDDEN : 4 * HIDDEN] = W2[P:]
    wts[:, 4 * HIDDEN : 4 * HIDDEN + DOUT] = W3[:P]
    wts[:, 4 * HIDDEN + DOUT : 4 * HIDDEN + 2 * DOUT] = W3[P:]
    return wts


def _build_c16(META, dstrelT):
    """fp16 consts row-block: iota ramp | dstrel."""
    gbounds, wparts, gchunks, NCH = META
    _, oh_batches = _oh_plan(gbounds, gchunks)
    CB_OH = max(top - base for base, top, _ in oh_batches)
    c16 = np.empty((P, CB_OH * W + NCH), np.float16)
    c16[:, 0 : CB_OH * W] = np.tile(
        np.arange(W, dtype=np.float16), CB_OH
    )[None, :]
    c16[:, CB_OH * W :] = dstrelT
    return c16


def _make_in_maps(META, per_core, b1, b2, b3, W1, W2, W3):
    consts = _build_consts(b1, b2, b3)
    wts = _build_wts(W1, W2, W3)
    return [
        {
            "attr8": pc["attr8"].view(np.uint8),
            "x16": pc["x16"],
            "c16": _build_c16(META, pc["dstrelT"]),
            "consts": consts,
            "wts": wts,
        }
        for pc in per_core
    ]


def kernel(x, edge_index, edge_attr, W1, b1, W2, b2, W3, b3):
    META, per_core, slot_node = _host_prep(x, edge_index, edge_attr)

    if META not in _prog_cache:
        _prog_cache[META] = _build_program(META)
    nc = _prog_cache[META]

    in_maps = _make_in_maps(
        META, per_core,
        np.asarray(b1, np.float32), np.asarray(b2, np.float32),
        np.asarray(b3, np.float32),
        np.asarray(W1, np.float32), np.asarray(W2, np.float32),
        np.asarray(W3, np.float32),
    )
    res = run_bass_kernel_spmd(nc, in_maps, core_ids=list(range(N_CORES)))

    out = np.empty((N_NODES, DOUT), np.float32)
    for c in range(N_CORES):
        o = res.results[c]["outT"].T.astype(np.float32)
        smap = slot_node[c]
        m = smap >= 0
        out[smap[m]] = o[m]
    return out


# revision 95
# speedup vs baseline: 1.6924x; 1.0038x over previous
"""NodeNet GNN message-passing kernel for 8 Trainium2 NeuronCores.

Strategy (per sharding hint): shard nodes across the 8 cores; partition
edges by destination node on the host so the scatter-mean is device-local.

v3 — fp8 edge stream + padding-minimizing packing + pipelined groups:
  - Host sorts each core's 12,500 nodes by descending edge count and sorts
    edges by destination; edge rows are pre-scaled by 1/count(dst) so the
    device segment-sum directly yields the mean, then cast to fp8 e4m3
    (end-to-end absmax error ~9e-3 vs the 2e-2 gate; the scatter-mean
    averages the quantization noise before the MLP sees it).
  - Windows hold up to W=16 node slots; a host DP over the rank-wise mean
    count profile picks window spans whose edge lists land near 64-edge
    half-chunk boundaries. Two half-windows in a group share one 128-row
    chunk (partitions 0:64 / 64:128). Each core packs nodes head-first and
    plugs boundary gaps with its smallest remaining nodes (tail fill);
    caps then shrink to the cross-core max usage (the fill replays
    identically), leaving ~5% edge padding. One shared SPMD chunk plan.
  - Per group (~32 windows, 512 node slots): ScalarE pre-expands dst-rel
    to a packed wide tile (a stride-0 broadcast operand would deny DVE its
    2x mode), VectorE builds the one-hot with ONE 2x is_equal against an
    iota ramp, and TensorE contracts chunk-by-chunk (fp8 lhsT x fp16 rhs,
    fp32 accumulate) into per-window slices of one PSUM bank tile.
  - The 3-layer MLP runs feature-major per group; PSUM evacuations are
    split ScalarE (mean, h1a, h2a, out) / VectorE (h1b, h2b) — chosen by
    sweep: in-order queue coupling, not busy balance, sets the cadence.
  - Groups are processed smallest-first (reversed plan order); outputs
    accumulate in one SBUF tile and are flushed as large slabs from the
    idle Pool engine's SWDGE queue, 2 groups behind, so stores can never
    head-of-line block the SP edge-stream queue.

Cost-model timeline (per core): ~97 us DMA (34 MB/core at 360 GB/s, 89%
occupancy), ACT ~92 us, PE ~68 us, DVE ~49 us; wall 108.4 us vs the
182.2 us session-start baseline.
"""

import numpy as np
import ml_dtypes

import concourse.bacc as bacc
import concourse.mybir as mybir
import concourse.tile as tile
from concourse.bass_utils import run_bass_kernel_spmd

P = 128                    # partitions / matmul contraction tile
D = 128                    # node & edge feature dim
HIDDEN = 256
DOUT = 128
N_NODES = 100000
N_CORES = 8
NPC_REAL = 12500           # real nodes per core
W = 16                     # node slots per window (one-hot width)

# marginal cost weights for the host packing DP (ns, from the TRN2 cost
# model): one 128-edge chunk costs DMA 45.5 + PE 6.7 + DVE 16.7; one
# window costs 16 node slots of MLP/DMA work
CHUNK_COST = 70.0
WINDOW_COST = 90.0
PLAN_SLACK = 0     # slots per window the DP leaves for tail fillers

_prog_cache: dict = {}

# engine assignment for the six PSUM evacuations (chain cadence tuning)
EVAC = {"mean": "A", "h1a": "A", "h1b": "V", "h2a": "A", "h2b": "V", "og": "A"}
EXP_ENG = "A"     # dstrel expand: "A" ScalarE, "P" Pool, "N" none (1x)
STORE_ENG = "P"   # slab stores: "P" Pool SWDGE, "S" SP queue
STORE_LAG = 1     # groups of lag before a finished slab is stored
STORE_SLAB = 1    # flush cadence in processed groups
OH_PAIR = 1       # processed groups sharing one expand+is_equal pair
ATTR_BUFS = 8
OH_BUFS = 5
PBIN_BUFS = 2
PO_BUFS = 2
ACT_BUFS = 8


def _oh_plan(gbounds, gchunks):
    """Processing order (reversed plan) and one-hot batches: OH_PAIR
    consecutive processed groups share one contiguous chunk span."""
    porder = list(range(len(gbounds)))[::-1]
    batches = []
    for i in range(0, len(porder), OH_PAIR):
        mem = porder[i : i + OH_PAIR]
        base = min(gchunks[g][0] for g in mem)
        top = max(gchunks[g][1] for g in mem)
        batches.append((base, top, tuple(mem)))
    return porder, batches

f32 = mybir.dt.float32
f16 = mybir.dt.float16
f8 = mybir.dt.float8e4

Relu = mybir.ActivationFunctionType.Relu
Ident = mybir.ActivationFunctionType.Identity


def _evac(nc, key, out, in_, bias, relu=True):
    """PSUM->SBUF evacuation with bias (+relu) on ACT or DVE per EVAC.
    Mode "S" splits the columns: ACT takes the first half, DVE the
    second, halving that chain stage's latency."""
    if EVAC[key] == "S":
        ncols = out.shape[-1]
        h = (ncols // 2 + 1) & ~1
        saved = dict(EVAC)
        try:
            EVAC[key] = "A"
            _evac(nc, key, out[:, :h], in_[:, :h], bias, relu)
            EVAC[key] = "V"
            _evac(nc, key, out[:, h:], in_[:, h:], bias, relu)
        finally:
            EVAC.update(saved)
        return
    if EVAC[key] == "A":
        if bias is None:
            nc.scalar.copy(out=out, in_=in_)
        else:
            nc.scalar.activation(out=out, in_=in_,
                                 func=(Relu if relu else Ident), bias=bias)
    else:
        if bias is None:
            nc.vector.tensor_copy(out=out, in_=in_)
        elif relu:
            nc.vector.tensor_scalar(
                out=out, in0=in_, scalar1=bias, scalar2=0.0,
                op0=mybir.AluOpType.add, op1=mybir.AluOpType.max,
            )
        else:
            nc.vector.tensor_scalar(
                out=out, in0=in_, scalar1=bias, scalar2=None,
                op0=mybir.AluOpType.add,
            )


def _group_plan(n_windows):
    """Group sizes in windows: small groups first (compute starts early),
    steady-state 32-window groups (512 node slots), aggressively tapered
    tail (the backlog drains at per-group chain latency, so the last
    chains must be short)."""
    # groups are PROCESSED in reversed plan order; both ends taper (fast
    # pipeline fill at the processing start, short chains at the drain)
    gpw = 512 // W  # windows per full 512-column group
    tail = [(3 * gpw) // 4, gpw // 2, gpw // 4]
    gsizes = list(tail[::-1])
    rem = n_windows - sum(gsizes) - sum(tail)
    while rem >= gpw:
        gsizes.append(gpw)
        rem -= gpw
    if rem:
        gsizes.append(rem)
    return gsizes + tail


def _build_program(META):
    """META = (gbounds, wparts, gchunks, NCH): per-group window bounds,
    per-window matmul parts (chunk col, partition lo, hi), per-group
    chunk ranges; identical across cores (SPMD)."""
    gbounds, wparts, gchunks, NCH = META
    n_windows = len(wparts)
    NPC = n_windows * W
    CBG_max = max(c1 - c0 for c0, c1 in gchunks)
    NW_max = max(w1 - w0 for w0, w1 in gbounds) * W
    porder, oh_batches = _oh_plan(gbounds, gchunks)
    CB_OH = max(top - base for base, top, _ in oh_batches)
    batch_of = {}
    for bi, (_, _, mem) in enumerate(oh_batches):
        for g in mem:
            batch_of[g] = bi

    nc = bacc.Bacc(None)
    attr8_d = nc.dram_tensor("attr8", [P, NCH * D], f8, kind="ExternalInput")
    x16_d = nc.dram_tensor("x16", [P, NPC], f16, kind="ExternalInput")
    # fp16 consts: iota ramp (CB_OH*W) | dstrel (NCH)
    c16_d = nc.dram_tensor("c16", [P, CB_OH * W + NCH], f16, kind="ExternalInput")
    consts_d = nc.dram_tensor("consts", [P, 5], f32, kind="ExternalInput")
    wts_d = nc.dram_tensor("wts", [P, 4 * HIDDEN + 2 * DOUT], f16,
                           kind="ExternalInput")
    outT_d = nc.dram_tensor("outT", [P, NPC], f16, kind="ExternalOutput")

    with tile.TileContext(nc) as tc:
        with (
            tc.tile_pool(name="const", bufs=1) as cpool,
            tc.tile_pool(name="attr", bufs=ATTR_BUFS) as apool,
            tc.tile_pool(name="x", bufs=4) as xpool,
            tc.tile_pool(name="oh", bufs=OH_BUFS) as ohpool,
            tc.tile_pool(name="dw", bufs=3) as dwpool,
            tc.tile_pool(name="acts", bufs=ACT_BUFS) as actpool,
            tc.tile_pool(name="pbin", bufs=PBIN_BUFS, space="PSUM") as pbin,
            tc.tile_pool(name="pmlp", bufs=1, space="PSUM") as pmlp,
            tc.tile_pool(name="ppo", bufs=PO_BUFS, space="PSUM") as ppo,
        ):
            cs = cpool.tile([P, 5], f32, tag="consts")
            ws = cpool.tile([P, 4 * HIDDEN + 2 * DOUT], f16, tag="wts")
            c16 = cpool.tile([P, CB_OH * W + NCH], f16, tag="c16")
            w1s_0 = ws[:, 0:HIDDEN]
            w1s_1 = ws[:, HIDDEN : 2 * HIDDEN]
            w2s_0 = ws[:, 2 * HIDDEN : 3 * HIDDEN]
            w2s_1 = ws[:, 3 * HIDDEN : 4 * HIDDEN]
            w3s_0 = ws[:, 4 * HIDDEN : 4 * HIDDEN + DOUT]
            w3s_1 = ws[:, 4 * HIDDEN + DOUT : 4 * HIDDEN + 2 * DOUT]
            b1s_0 = cs[:, 0:1]
            b1s_1 = cs[:, 1:2]
            b2s_0 = cs[:, 2:3]
            b2s_1 = cs[:, 3:4]
            b3s = cs[:, 4:5]
            it16 = c16[:, 0 : CB_OH * W]
            dstrel_s = c16[:, CB_OH * W : CB_OH * W + NCH]
            oall = cpool.tile([P, NPC], f16, tag="oall")

            # deferred slab stores: flush finished output columns with a
            # lag so the store never races its own evacuations
            SLAB = STORE_SLAB
            LAG = STORE_LAG
            store_dma = (nc.gpsimd.dma_start if STORE_ENG == "P"
                         else nc.sync.dma_start)

            def build_oh(bi):
                """One-hot for batch bi (OH_PAIR processed groups' chunk
                span) — depends only on c16. ScalarE expands dstrel to a
                packed wide tile first: a broadcast (stride-0) operand
                would deny the DVE is_equal its 2x mode."""
                bco0, bco1, _ = oh_batches[bi]
                bCB = bco1 - bco0
                oh = ohpool.tile([P, CB_OH * W], f16, tag="oh")
                if EXP_ENG == "N":
                    # plain 1x is_equal with the broadcast operand (all DVE)
                    nc.vector.tensor_tensor(
                        out=oh[:, : bCB * W].rearrange("p (c m) -> p c m", m=W),
                        in0=dstrel_s[:, bco0:bco1].to_broadcast([P, bCB, W]),
                        in1=it16[:, : bCB * W].rearrange("p (c m) -> p c m", m=W),
                        op=mybir.AluOpType.is_equal,
                    )
                    return oh
                dw = dwpool.tile([P, CB_OH * W], f16, tag="dw")
                exp_copy = {"A": nc.scalar.copy, "P": nc.gpsimd.tensor_copy,
                            "V": nc.vector.tensor_copy}[EXP_ENG]
                exp_copy(
                    out=dw[:, : bCB * W].rearrange("p (c m) -> p c m", m=W),
                    in_=dstrel_s[:, bco0:bco1].to_broadcast([P, bCB, W]),
                )
                nc.vector.tensor_tensor(
                    out=oh[:, : bCB * W],
                    in0=dw[:, : bCB * W],
                    in1=it16[:, : bCB * W],
                    op=mybir.AluOpType.is_equal,
                )
                return oh

            # process groups smallest-first (reversed plan order): the big
            # groups' delivery cadence exceeds their compute cadence, so the
            # compute backlog shrinks toward the end and the drain collapses
            # to roughly one group's chain latency
            oh_cache = {}   # batch index -> oh tile
            flush_hi = NPC
            first = True
            for gi, g in enumerate(porder):
                w0, w1 = gbounds[g]
                NW = (w1 - w0) * W
                n0 = w0 * W
                co0, co1 = gchunks[g]
                CBg = co1 - co0
                if CBg == 0:
                    # no core placed any node here (shrunk spare windows)
                    continue

                at8 = apool.tile([P, CBG_max * D], f8, tag="attr")
                nc.sync.dma_start(
                    out=at8[:, : CBg * D], in_=attr8_d[:, co0 * D : co1 * D]
                )
                if first:
                    first = False
                    nc.sync.dma_start(out=c16[:], in_=c16_d[:, :])
                    nc.sync.dma_start(out=cs[:], in_=consts_d[:, :])
                    nc.sync.dma_start(out=ws[:], in_=wts_d[:, :])
                xg = xpool.tile([P, NW_max], f16, tag="x")
                nc.sync.dma_start(out=xg[:, :NW], in_=x16_d[:, n0 : n0 + NW])
                if gi >= SLAB + LAG and gi % SLAB == 0:
                    f0 = gbounds[porder[gi - LAG]][0] * W
                    store_dma(
                        out=outT_d[:, f0:flush_hi], in_=oall[:, f0:flush_hi]
                    )
                    flush_hi = f0

                bi = batch_of[g]
                if bi not in oh_cache:
                    oh_cache = {bi: build_oh(bi)}
                oh = oh_cache[bi]
                ob = oh_batches[bi][0]  # batch chunk base

                pm = pbin.tile([P, NW_max], f32, tag="pm")
                for w in range(w0, w1):
                    parts = wparts[w]
                    sw = w - w0
                    for i, (c, p0, p1) in enumerate(parts):
                        lc = c - co0
                        lo = c - ob
                        nc.tensor.matmul(
                            out=pm[:, sw * W : (sw + 1) * W],
                            lhsT=at8[p0:p1, lc * D : (lc + 1) * D],
                            rhs=oh[p0:p1, lo * W : (lo + 1) * W],
                            start=(i == 0),
                            stop=(i == len(parts) - 1),
                        )
                mean_g = actpool.tile([P, NW_max], f16, tag="mean_g")
                _evac(nc, "mean", mean_g[:, :NW], pm[:, :NW], None)

                # --- MLP over this group (feature-major) ---
                ph1a = pmlp.tile([P, NW_max], f32, tag="h1a")
                ph1b = pmlp.tile([P, NW_max], f32, tag="h1b")
                nc.tensor.matmul(out=ph1a[:, :NW], lhsT=w1s_0[:, 0:P],
                                 rhs=xg[:, :NW], start=True, stop=False)
                nc.tensor.matmul(out=ph1a[:, :NW], lhsT=w1s_1[:, 0:P],
                                 rhs=mean_g[:, :NW], start=False, stop=True)
                nc.tensor.matmul(out=ph1b[:, :NW], lhsT=w1s_0[:, P:HIDDEN],
                                 rhs=xg[:, :NW], start=True, stop=False)
                nc.tensor.matmul(out=ph1b[:, :NW], lhsT=w1s_1[:, P:HIDDEN],
                                 rhs=mean_g[:, :NW], start=False, stop=True)
                h1a = actpool.tile([P, NW_max], f16, tag="h1a_s")
                h1b = actpool.tile([P, NW_max], f16, tag="h1b_s")
                # paired evacuations on different engines run in parallel
                # instead of back-to-back on one in-order queue
                _evac(nc, "h1a", h1a[:, :NW], ph1a[:, :NW], b1s_0[:, 0:1])
                _evac(nc, "h1b", h1b[:, :NW], ph1b[:, :NW], b1s_1[:, 0:1])

                ph2a = pmlp.tile([P, NW_max], f32, tag="h2a")
                ph2b = pmlp.tile([P, NW_max], f32, tag="h2b")
                nc.tensor.matmul(out=ph2a[:, :NW], lhsT=w2s_0[:, 0:P],
                                 rhs=h1a[:, :NW], start=True, stop=False)
                nc.tensor.matmul(out=ph2a[:, :NW], lhsT=w2s_1[:, 0:P],
                                 rhs=h1b[:, :NW], start=False, stop=True)
                nc.tensor.matmul(out=ph2b[:, :NW], lhsT=w2s_0[:, P:HIDDEN],
                                 rhs=h1a[:, :NW], start=True, stop=False)
                nc.tensor.matmul(out=ph2b[:, :NW], lhsT=w2s_1[:, P:HIDDEN],
                                 rhs=h1b[:, :NW], start=False, stop=True)
                h2a = actpool.tile([P, NW_max], f16, tag="h2a_s")
                h2b = actpool.tile([P, NW_max], f16, tag="h2b_s")
                _evac(nc, "h2a", h2a[:, :NW], ph2a[:, :NW], b2s_0[:, 0:1])
                _evac(nc, "h2b", h2b[:, :NW], ph2b[:, :NW], b2s_1[:, 0:1])

                po = ppo.tile([P, NW_max], f32, tag="po")
                nc.tensor.matmul(out=po[:, :NW], lhsT=w3s_0[:],
                                 rhs=h2a[:, :NW], start=True, stop=False)
                nc.tensor.matmul(out=po[:, :NW], lhsT=w3s_1[:],
                                 rhs=h2b[:, :NW], start=False, stop=True)
                _evac(nc, "og", oall[:, n0 : n0 + NW], po[:, :NW],
                      b3s[:, 0:1], relu=False)

            store_dma(
                out=outT_d[:, 0:flush_hi], in_=oall[:, 0:flush_hi]
            )

    nc.finalize()
    return nc


def _pack_windows(profile):
    """DP over node ranks: choose window boundaries (<=W nodes each) to
    minimize chunk+window cost. profile = descending per-rank edge-count
    upper bound. Returns list of per-window node spans (r0, r1)."""
    n = len(profile)
    csum = np.zeros(n + 1, np.float64)
    csum[1:] = np.cumsum(profile)
    INF = float("inf")
    dp = np.full(n + 1, INF)
    prev = np.zeros(n + 1, np.int32)
    dp[0] = 0.0
    wplan = W - PLAN_SLACK  # reserve slots per window for tail fillers
    for r1 in range(1, n + 1):
        best = INF
        barg = r1 - 1
        for r0 in range(max(0, r1 - wplan), r1):
            if dp[r0] == INF:
                continue
            e = csum[r1] - csum[r0]
            # windows share tail chunks in 64-edge halves, so capacity is
            # 64-granular at half the per-chunk cost
            cost = dp[r0] + CHUNK_COST * 0.5 * max((e + 63) // 64, 1) + WINDOW_COST
            if cost < best:
                best = cost
                barg = r0
        dp[r1] = best
        prev[r1] = barg
    spans = []
    r = n
    while r > 0:
        spans.append((int(prev[r]), r))
        r = int(prev[r])
    spans.reverse()
    return spans


def _fill(cnts, cap, n_windows):
    """Head+tail fill: place descending-count nodes (head pointer) into
    windows (<=W slots, cap[j] edge budget); when the next head node
    overflows the remaining budget, plug the gap with the globally
    smallest remaining nodes (tail pointer), which fit nearly exactly.
    Returns placement arrays + per-window usage."""
    n = len(cnts)
    plorder = np.empty(n, np.int64)
    win_of_place = np.empty(n, np.int64)
    idx_of_place = np.empty(n, np.int64)
    used = np.zeros(n_windows, np.int64)
    nplace = np.zeros(n_windows, np.int64)
    hp = 0
    tp = n - 1
    p = 0
    for j in range(n_windows):
        if hp > tp:
            break
        e_used = 0
        slots = 0

        def place(rk, e):
            nonlocal p, slots
            plorder[p] = rk
            win_of_place[p] = j
            idx_of_place[p] = slots
            slots += 1
            p += 1
            return e + cnts[rk]

        while slots < W and hp <= tp and e_used + cnts[hp] <= cap[j]:
            e_used = place(hp, e_used)
            hp += 1
        # gap fill with the smallest remaining nodes
        while slots < W and hp <= tp and e_used + cnts[tp] <= cap[j]:
            e_used = place(tp, e_used)
            tp -= 1
        used[j] = e_used
        nplace[j] = slots
    assert p == n, "core overflowed the window plan"
    return plorder, win_of_place, idx_of_place, used, nplace


def _host_prep(x, edge_index, edge_attr):
    """Sort/scale/pad edges; returns (META, per-core input arrays,
    per-core slot->global-node maps)."""
    col = np.asarray(edge_index)[1].astype(np.int64)
    x = np.asarray(x, dtype=np.float32)
    counts = np.bincount(col, minlength=N_NODES).astype(np.int64)
    scale = (1.0 / np.maximum(counts, 1)).astype(np.float32)

    eorder = np.argsort(col, kind="stable")
    col_s = col[eorder]
    attr_s = np.asarray(edge_attr, dtype=np.float32)[eorder]
    attr_s = attr_s * scale[col_s][:, None]
    attr_s8 = attr_s.astype(ml_dtypes.float8_e4m3)
    # edge start offset of each node in the dst-sorted arrays
    estart = np.zeros(N_NODES + 1, np.int64)
    estart[1:] = np.cumsum(counts)

    # per-core descending-count node order + rank-wise max profile
    nodeperm = np.empty((N_CORES, NPC_REAL), np.int64)   # rank -> local node
    cnt_sorted = np.empty((N_CORES, NPC_REAL), np.int64)
    for c in range(N_CORES):
        cnt_c = counts[c * NPC_REAL : (c + 1) * NPC_REAL]
        p = np.argsort(-cnt_c, kind="stable")
        nodeperm[c] = p
        cnt_sorted[c] = cnt_c[p]
    # plan on the rank-wise mean profile; per-core greedy fill spills
    # overflow forward, spare windows at the end absorb the tail
    profile = cnt_sorted.mean(axis=0)

    spans = _pack_windows(profile)
    n_spare = 32
    spans = spans + [(NPC_REAL, NPC_REAL)] * n_spare
    n_windows = len(spans)

    # planned per-window capacity (64-edge granular)
    cap0 = np.zeros(n_windows, np.int64)
    for j, (r0, r1) in enumerate(spans):
        s = float(profile[r0:r1].sum())
        cap0[j] = max(int(np.ceil(s / 64)) * 64, 64)

    # fill every core against the planned caps, then shrink each cap to
    # the cross-core max usage (the fill provably replays identically
    # under shrunk caps, so the placements stay valid)
    fills = [_fill(cnt_sorted[c], cap0, n_windows) for c in range(N_CORES)]
    used_max = np.max([f[3] for f in fills], axis=0)
    placed_any = np.max([f[4] for f in fills], axis=0) > 0
    cap = ((used_max + 63) // 64) * 64
    cap[placed_any & (cap == 0)] = 64

    # per-window chunks: full chunks + optional shared half chunk (two
    # half-windows in a group share one 128-row chunk, first in
    # partitions 0:64, second in 64:128)
    fullc = cap // 128
    is_half = (cap % 128) > 0

    gsizes = _group_plan(n_windows)
    gbounds = []
    w = 0
    for gs in gsizes:
        gbounds.append((w, w + gs))
        w += gs

    # chunk allocation + half pairing (within groups, so group DMA slices
    # stay contiguous)
    wparts = [[] for _ in range(n_windows)]
    gchunks = []
    colc = 0
    for w0, w1 in gbounds:
        co0 = colc
        pend = None
        for j in range(w0, w1):
            for _ in range(fullc[j]):
                wparts[j].append((colc, 0, 128))
                colc += 1
            if is_half[j]:
                if pend is None:
                    pend = colc
                    wparts[j].append((colc, 0, 64))
                    colc += 1
                else:
                    wparts[j].append((pend, 64, 128))
                    pend = None
        gchunks.append((co0, colc))
    NCH = colc
    E_pad = NCH * P
    NPC = n_windows * W

    capstart = np.zeros(n_windows + 1, np.int64)
    capstart[1:] = np.cumsum(cap)
    pos_all = np.empty(int(cap.sum()), np.int64)
    for j in range(n_windows):
        o = capstart[j]
        for c, p0, p1 in wparts[j]:
            pos_all[o : o + p1 - p0] = np.arange(c * P + p0, c * P + p1)
            o += p1 - p0

    META = (
        tuple(gbounds),
        tuple(tuple(p) for p in wparts),
        tuple(gchunks),
        NCH,
    )

    per_core = []
    slot_node = []  # per core: slot -> global node id (-1 empty)
    for c in range(N_CORES):
        cnts = cnt_sorted[c]
        plorder, win_of_place, idx_of_place, _, _ = fills[c]
        cnts_p = cnts[plorder]
        newwin = np.ones(NPC_REAL, bool)
        newwin[1:] = win_of_place[1:] != win_of_place[:-1]
        pre = np.cumsum(cnts_p) - cnts_p
        wstart = np.maximum.accumulate(np.where(newwin, pre, -1))
        prefix_in_win = pre - wstart
        rank_base = capstart[win_of_place] + prefix_in_win

        # edges in placement order
        lnode = nodeperm[c][plorder]           # placement -> local node
        gnode = lnode + c * NPC_REAL
        src0 = estart[gnode]                   # first edge (dst-sorted)
        total = int(cnts_p.sum())
        src_idx = np.concatenate(
            [np.arange(src0[i], src0[i] + cnts_p[i]) for i in range(NPC_REAL)]
        ) if total else np.empty(0, np.int64)
        within = np.arange(total) - np.repeat(pre, cnts_p)
        edest = pos_all[np.repeat(rank_base, cnts_p) + within]

        attr_pad = np.zeros((E_pad, D), ml_dtypes.float8_e4m3)
        attr_pad[edest] = attr_s8[src_idx]
        attr8 = np.ascontiguousarray(
            attr_pad.reshape(NCH, P, D).transpose(1, 0, 2).reshape(P, NCH * D)
        )

        dstrel = np.full((E_pad,), 200.0, np.float16)
        dstrel[edest] = np.repeat(idx_of_place, cnts_p).astype(np.float16)
        dstrelT = np.ascontiguousarray(dstrel.reshape(NCH, P).T)

        # node features + slot map
        slot = win_of_place * W + idx_of_place
        smap = np.full(NPC, -1, np.int64)
        smap[slot] = gnode
        xT = np.zeros((NPC, D), np.float16)
        xT[slot] = x[gnode].astype(np.float16)
        xT = np.ascontiguousarray(xT.T)

        per_core.append({"attr8": attr8, "dstrelT": dstrelT, "x16": xT})
        slot_node.append(smap)
    return META, per_core, slot_node


def _build_consts(b1, b2, b3):
    consts = np.zeros((P, 5), np.float32)
    consts[:, 0] = b1[:P]
    consts[:, 1] = b1[P:]
    consts[:, 2] = b2[:P]
    consts[:, 3] = b2[P:]
    consts[:, 4] = b3
    return consts


def _build_wts(W1, W2, W3):
    wts = np.empty((P, 4 * HIDDEN + 2 * DOUT), np.float16)
    wts[:, 0:HIDDEN] = W1[:P]
    wts[:, HIDDEN : 2 * HIDDEN] = W1[P:]
    wts[:, 2 * HIDDEN : 3 * HIDDEN] = W2[:P]
    wts[:, 3 * HIDDEN : 4 * HIDDEN] = W2[P:]
    wts[:, 4 * HIDDEN : 4 * HIDDEN + DOUT] = W3[:P]
    wts[:, 4 * HIDDEN + DOUT : 4 * HIDDEN + 2 * DOUT] = W3[P:]
    return wts


def _build_c16(META, dstrelT):
    """fp16 consts row-block: iota ramp | dstrel."""
    gbounds, wparts, gchunks, NCH = META
    _, oh_batches = _oh_plan(gbounds, gchunks)
    CB_OH = max(top - base for base, top, _ in oh_batches)
    c16 = np.empty((P, CB_OH * W + NCH), np.float16)
    c16[:, 0 : CB_OH * W] = np.tile(
        np.arange(W, dtype=np.float16), CB_OH
    )[None, :]
    c16[:, CB_OH * W :] = dstrelT
    return c16


def _make_in_maps(META, per_core, b1, b2, b3, W1, W2, W3):
    consts = _build_consts(b1, b2, b3)
    wts = _build_wts(W1, W2, W3)
    return [
        {
            "attr8": pc["attr8"].view(np.uint8),
            "x16": pc["x16"],
            "c16": _build_c16(META, pc["dstrelT"]),
            "consts": consts,
            "wts": wts,
        }
        for pc in per_core
    ]


def kernel(x, edge_index, edge_attr, W1, b1, W2, b2, W3, b3):
    META, per_core, slot_node = _host_prep(x, edge_index, edge_attr)

    if META not in _prog_cache:
        _prog_cache[META] = _build_program(META)
    nc = _prog_cache[META]

    in_maps = _make_in_maps(
        META, per_core,
        np.asarray(b1, np.float32), np.asarray(b2, np.float32),
        np.asarray(b3, np.float32),
        np.asarray(W1, np.float32), np.asarray(W2, np.float32),
        np.asarray(W3, np.float32),
    )
    res = run_bass_kernel_spmd(nc, in_maps, core_ids=list(range(N_CORES)))

    out = np.empty((N_NODES, DOUT), np.float32)
    for c in range(N_CORES):
        o = res.results[c]["outT"].T.astype(np.float32)
        smap = slot_node[c]
        m = smap >= 0
        out[smap[m]] = o[m]
    return out
